# revision 1
# baseline (speedup 1.0000x reference)
"""Trainium2 Bass kernel for an MLP flow-matching GNN (message passing).

Strategy (8 NeuronCores, SPMD):
  - Host: sort edges by destination node, partition nodes (and their incident
    edges) into 8 contiguous ranges, pack whole-node edge groups into
    512-edge / 128-node blocks, precompute gather tables of PRE-PROJECTED
    node features (H1d = h @ W1a, H1s = h @ W1b) so the edge phase needs no
    per-edge weight matmul before the nonlinearity.
  - Device, per layer:
      edge phase:  indirect-DMA gather of projected rows -> PSUM assembly
                   (identity-matmul adds + K=4 matmul for [relpos,1] @ [W1c;b1])
                   -> SiLU -> PE transpose -> edge-MLP2 matmul -> SiLU ->
                   segment-sum via matmul against an is_equal-built indicator
                   matrix -> indirect-DMA scatter of per-block node sums.
      node phase:  node MLP + residual + LayerNorm on the core's own node
                   slice; also produces the next layer's projected tables.
      comm:        one AllGather of the projected tables.
  - Host: final projection + MSE loss (tiny).
"""

import numpy as np
import ml_dtypes

BF16 = ml_dtypes.bfloat16
EPS = 1e-5
NCORES = 8
P = 128          # partition width / hidden size (H must equal 128)
BLK_E = 512      # edges per block (4 panels of 128)
BLK_N = 128      # max distinct destination nodes per block
PAD_SLOT = 1 << 30


def _silu(x):
    return x * (1.0 / (1.0 + np.exp(-x)))


# ----------------------------------------------------------------------------
# Host-side preprocessing
# ----------------------------------------------------------------------------

def _pack_core(dst_loc, src_glob, rel, nv, nblk=None):
    """Pack one core's (dst-sorted, local-dst) edges into blocks.

    Returns dict of per-core device arrays. dst_loc in [0, nv); edges sorted
    by dst_loc. rel is [ec, 3] f32 (x_t[dst]-x_t[src]).
    """
    ec = dst_loc.shape[0]
    deg = np.bincount(dst_loc, minlength=nv)
    # greedy whole-node packing into (<=BLK_E edges, <=BLK_N nodes) blocks
    blocks = []  # (node_start, node_cnt, edge_start, edge_cnt)
    n0 = 0
    e0 = 0
    cur_n = 0
    cur_e = 0
    for v in range(nv):
        d = int(deg[v])
        assert d <= BLK_E, f"node degree {d} exceeds block capacity"
        if cur_n + 1 > BLK_N or cur_e + d > BLK_E:
            blocks.append((n0, cur_n, e0, cur_e))
            n0 += cur_n
            e0 += cur_e
            cur_n = 0
            cur_e = 0
        cur_n += 1
        cur_e += d
    blocks.append((n0, cur_n, e0, cur_e))
    assert n0 + cur_n == nv and e0 + cur_e == ec

    nb = len(blocks)
    if nblk is None:
        nblk = nb
    assert nb <= nblk
    ne = nblk * BLK_E

    dstidx = np.zeros(ne, np.int32)
    srcidx = np.zeros(ne, np.int32)
    dstrel = np.full(ne, -1, np.int32)
    rel4 = np.zeros((nblk, 4, BLK_E), np.float32)
    slots = np.broadcast_to(nv + np.arange(P, dtype=np.int32)[:, None],
                            (P, nblk)).copy()

    for b, (bn0, bcnt, bes, bec) in enumerate(blocks):
        sl = slice(b * BLK_E, b * BLK_E + bec)
        d = dst_loc[bes:bes + bec]
        s = src_glob[bes:bes + bec]
        # combined gather-table layout: core c contributes rows
        # [c*2nv, c*2nv+nv) = H1d slice, [c*2nv+nv, c*2nv+2nv) = H1s slice
        dstidx[sl] = d  # local for now; caller offsets to global table rows
        srcidx[sl] = s
        dstrel[sl] = d - bn0
        rel4[b, :3, :bec] = rel[bes:bes + bec].T
        rel4[b, 3, :bec] = 1.0
        if bcnt > 0:
            slots[:bcnt, b] = bn0 + np.arange(bcnt, dtype=np.int32)

    return {
        "blocks": blocks,
        "dstidx": dstidx,
        "srcidx": srcidx,
        "dstrel": dstrel,
        "rel4": rel4,
        "slots": slots,
        "nb": nb,
    }


def _preprocess(inputs):
    """All host-side math and index construction. Returns (geom, per_core, host)."""
    pos0 = np.asarray(inputs["pos0"], np.float32)
    pos1 = np.asarray(inputs["pos1"], np.float32)
    z = np.asarray(inputs["z"], np.float32)
    t = np.asarray(inputs["t"], np.float32)
    edge_index = np.asarray(inputs["edge_index"])
    batch = np.asarray(inputs["batch"])
    ew1 = np.asarray(inputs["ew1"], np.float32)
    eb1 = np.asarray(inputs["eb1"], np.float32)
    ew2 = np.asarray(inputs["ew2"], np.float32)
    eb2 = np.asarray(inputs["eb2"], np.float32)
    nw1 = np.asarray(inputs["nw1"], np.float32)
    nb1 = np.asarray(inputs["nb1"], np.float32)
    nw2 = np.asarray(inputs["nw2"], np.float32)
    nb2 = np.asarray(inputs["nb2"], np.float32)
    ln_g = np.asarray(inputs["ln_g"], np.float32)
    ln_b = np.asarray(inputs["ln_b"], np.float32)

    V = pos0.shape[0]
    L = ew1.shape[0]
    H = ew1.shape[2]
    assert H == P
    nv = V // NCORES
    assert nv * NCORES == V

    ts = float(t[0])
    x_t = (1.0 - ts) * pos0 + ts * pos1
    target = pos1 - pos0

    te_w1 = np.asarray(inputs["te_w1"], np.float32)
    te_b1 = np.asarray(inputs["te_b1"], np.float32)
    te_w2 = np.asarray(inputs["te_w2"], np.float32)
    te_b2 = np.asarray(inputs["te_b2"], np.float32)
    cp_w = np.asarray(inputs["cp_w"], np.float32)
    cp_b = np.asarray(inputs["cp_b"], np.float32)

    t_emb = _silu(np.array([[ts]], np.float32) @ te_w1 + te_b1) @ te_w2 + te_b2
    h0 = np.concatenate(
        [z[batch], np.broadcast_to(t_emb, (V, t_emb.shape[1]))], axis=1
    ) @ cp_w + cp_b  # [V, H] f32

    # layer-0 projected gather table, per-core interleaved [d-slice; s-slice]
    H1d0 = h0 @ ew1[0, :H]
    H1s0 = h0 @ ew1[0, H:2 * H]
    h1full0 = np.empty((2 * V, H), np.float32)
    for c in range(NCORES):
        h1full0[c * 2 * nv: c * 2 * nv + nv] = H1d0[c * nv:(c + 1) * nv]
        h1full0[c * 2 * nv + nv: (c + 1) * 2 * nv] = H1s0[c * nv:(c + 1) * nv]

    # edges sorted by destination, split at node-range boundaries
    src_g = edge_index[0].astype(np.int64)
    dst_g = edge_index[1].astype(np.int64)
    order = np.argsort(dst_g, kind="stable")
    dst_s = dst_g[order]
    src_s = src_g[order]
    bounds = np.searchsorted(dst_s, np.arange(0, V + 1, nv))

    rel_all = x_t[dst_s] - x_t[src_s]  # [E, 3]

    packs = []
    for c in range(NCORES):
        e0, e1 = int(bounds[c]), int(bounds[c + 1])
        packs.append(_pack_core(
            (dst_s[e0:e1] - c * nv).astype(np.int64), src_s[e0:e1],
            rel_all[e0:e1], nv))
    nblk = max(p["nb"] for p in packs)

    per_core = []
    for c, pk in enumerate(packs):
        e0, e1 = int(bounds[c]), int(bounds[c + 1])
        pk = _pack_core(
            (dst_s[e0:e1] - c * nv).astype(np.int64), src_s[e0:e1],
            rel_all[e0:e1], nv, nblk=nblk)
        ne = nblk * BLK_E
        nbcol = nblk * 4
        # to combined-table global rows
        dl = pk["dstidx"].astype(np.int64) + c * nv  # back to global node id
        sg = pk["srcidx"].astype(np.int64)
        drow = (dl // nv) * 2 * nv + (dl % nv)
        srow = (sg // nv) * 2 * nv + nv + (sg % nv)
        per_core.append({
            "dstidx": drow.astype(np.int32).reshape(nbcol, P).T.copy(),
            "srcidx": srow.astype(np.int32).reshape(nbcol, P).T.copy(),
            "dstrel": pk["dstrel"].reshape(nbcol, P).T.copy(),
            "slots": pk["slots"],
            "rel4": pk["rel4"].astype(BF16),
        })

    # device weight layouts (concat layers along free dim)
    w1c4 = np.concatenate(
        [np.concatenate([ew1[l, 2 * H:], eb1[l][None, :]], 0) for l in range(L)],
        axis=1).astype(BF16)                                     # [4, L*H]
    w1a = np.concatenate([ew1[l, :H] for l in range(L)], 1).astype(BF16)
    w1b = np.concatenate([ew1[l, H:2 * H] for l in range(L)], 1).astype(BF16)
    w2 = np.concatenate([ew2[l] for l in range(L)], 1).astype(BF16)
    b2bc = np.concatenate(
        [np.broadcast_to(eb2[l], (P, H)) for l in range(L)], 1).astype(np.float32)
    nw1h = np.concatenate([nw1[l, :H] for l in range(L)], 1).astype(BF16)
    nw1a = np.concatenate([nw1[l, H:] for l in range(L)], 1).astype(BF16)
    nw2c = np.concatenate([nw2[l] for l in range(L)], 1).astype(BF16)
    nb1c = nb1.T.astype(np.float32).copy()                       # [H, L]
    nb2bc = np.concatenate(
        [np.broadcast_to(nb2[l], (P, H)) for l in range(L)], 1).astype(np.float32)
    lngbc = np.concatenate(
        [np.broadcast_to(ln_g[l], (P, H)) for l in range(L)], 1).astype(np.float32)
    lnbbc = np.concatenate(
        [np.broadcast_to(ln_b[l], (P, H)) for l in range(L)], 1).astype(np.float32)

    ident = np.eye(P, dtype=BF16)
    iota = np.tile(np.arange(P, dtype=np.int32), (P, 1))

    geom = dict(V=V, L=L, H=H, nv=nv, nblk=nblk)
    weights = dict(w1c4=w1c4, w1a=w1a, w1b=w1b, w2=w2, b2bc=b2bc,
                   nw1h=nw1h, nw1a=nw1a, nw2=nw2c, nb1c=nb1c, nb2bc=nb2bc,
                   lngbc=lngbc, lnbbc=lnbbc, ident=ident, iota=iota)
    host = dict(h1full0=h1full0.astype(BF16), h0=h0, target=target,
                op_w=np.asarray(inputs["op_w"], np.float32),
                op_b=np.asarray(inputs["op_b"], np.float32))
    return geom, per_core, weights, host


# ----------------------------------------------------------------------------
# Device program
# ----------------------------------------------------------------------------

SILU_DECOMPOSED = False  # sim has no Silu table; set True for CoreSim runs


def _build_program(geom):
    import concourse.bass as bass
    import concourse.bacc as bacc
    import concourse.mybir as mybir
    import concourse.tile as tile

    dt = mybir.dt
    AF = mybir.ActivationFunctionType
    ALU = mybir.AluOpType
    IOA = bass.IndirectOffsetOnAxis

    V, L, nv, nblk = geom["V"], geom["L"], geom["nv"], geom["nblk"]
    nbcol = nblk * 4
    NW = (nv + P - 1) // P  # node windows per core

    nc = bacc.Bacc(num_devices=NCORES)

    # ---- I/O ----
    h1f0 = nc.declare_dram_parameter("h1full0", [2 * V, P], dt.bfloat16, isOutput=False)
    h0own = nc.declare_dram_parameter("h0own", [nv, P], dt.bfloat16, isOutput=False)
    dstidx_d = nc.declare_dram_parameter("dstidx", [P, nbcol], dt.int32, isOutput=False)
    srcidx_d = nc.declare_dram_parameter("srcidx", [P, nbcol], dt.int32, isOutput=False)
    dstrel_d = nc.declare_dram_parameter("dstrel", [P, nbcol], dt.int32, isOutput=False)
    slots_d = nc.declare_dram_parameter("slots", [P, nblk], dt.int32, isOutput=False)
    rel4_d = nc.declare_dram_parameter("rel4", [nblk, 4, BLK_E], dt.bfloat16, isOutput=False)
    w1c4_d = nc.declare_dram_parameter("w1c4", [4, L * P], dt.bfloat16, isOutput=False)
    w1a_d = nc.declare_dram_parameter("w1a", [P, L * P], dt.bfloat16, isOutput=False)
    w1b_d = nc.declare_dram_parameter("w1b", [P, L * P], dt.bfloat16, isOutput=False)
    w2_d = nc.declare_dram_parameter("w2", [P, L * P], dt.bfloat16, isOutput=False)
    b2bc_d = nc.declare_dram_parameter("b2bc", [P, L * P], dt.float32, isOutput=False)
    nw1h_d = nc.declare_dram_parameter("nw1h", [P, L * P], dt.bfloat16, isOutput=False)
    nw1a_d = nc.declare_dram_parameter("nw1a", [P, L * P], dt.bfloat16, isOutput=False)
    nw2_d = nc.declare_dram_parameter("nw2", [P, L * P], dt.bfloat16, isOutput=False)
    nb1c_d = nc.declare_dram_parameter("nb1c", [P, L], dt.float32, isOutput=False)
    nb2bc_d = nc.declare_dram_parameter("nb2bc", [P, L * P], dt.float32, isOutput=False)
    lngbc_d = nc.declare_dram_parameter("lngbc", [P, L * P], dt.float32, isOutput=False)
    lnbbc_d = nc.declare_dram_parameter("lnbbc", [P, L * P], dt.float32, isOutput=False)
    ident_d = nc.declare_dram_parameter("ident", [P, P], dt.bfloat16, isOutput=False)
    iota_d = nc.declare_dram_parameter("iota", [P, P], dt.int32, isOutput=False)
    hout_d = nc.declare_dram_parameter("hout", [nv, P], dt.float32, isOutput=True)

    # ---- internal DRAM ----
    agg_hbm = nc.dram_tensor("agg_hbm", [nv + P, P], dt.bfloat16)
    h1own = [nc.dram_tensor(f"h1own{l}", [2 * nv, P], dt.bfloat16)
             for l in range(L - 1)]
    h1full = [nc.dram_tensor(f"h1full{l + 1}", [2 * V, P], dt.bfloat16,
                             addr_space="Shared") for l in range(L - 1)]
    hown = [nc.dram_tensor(f"hown{l + 1}", [nv, P], dt.bfloat16)
            for l in range(L - 1)]

    groups = [list(range(NCORES))]

    with tile.TileContext(nc) as tc:
        with (
            tc.tile_pool(name="const", bufs=1) as cpool,
            tc.tile_pool(name="gather", bufs=3) as gpool,
            tc.tile_pool(name="work", bufs=3) as wpool,
            tc.tile_pool(name="small", bufs=4) as spool,
            tc.tile_pool(name="pm1", bufs=2, space="PSUM") as pm1,
            tc.tile_pool(name="pm1t", bufs=2, space="PSUM") as pm1t,
            tc.tile_pool(name="pm2", bufs=2, space="PSUM") as pm2,
            tc.tile_pool(name="pagg", bufs=2, space="PSUM") as pagg,
        ):
            def cload(src, shape, dtype, tag):
                t_ = cpool.tile(shape, dtype, tag=tag)
                nc.sync.dma_start(out=t_[:], in_=src[:, :])
                return t_

            identsb = cload(ident_d, [P, P], dt.bfloat16, "ident")
            iotasb = cload(iota_d, [P, P], dt.int32, "iota")
            dstidxsb = cload(dstidx_d, [P, nbcol], dt.int32, "dstidx")
            srcidxsb = cload(srcidx_d, [P, nbcol], dt.int32, "srcidx")
            dstrelsb = cload(dstrel_d, [P, nbcol], dt.int32, "dstrel")
            slotsb = cload(slots_d, [P, nblk], dt.int32, "slots")
            w1c4sb = cload(w1c4_d, [4, L * P], dt.bfloat16, "w1c4")
            w1asb = cload(w1a_d, [P, L * P], dt.bfloat16, "w1a")
            w1bsb = cload(w1b_d, [P, L * P], dt.bfloat16, "w1b")
            w2sb = cload(w2_d, [P, L * P], dt.bfloat16, "w2")
            b2bcsb = cload(b2bc_d, [P, L * P], dt.float32, "b2bc")
            nw1hsb = cload(nw1h_d, [P, L * P], dt.bfloat16, "nw1h")
            nw1asb = cload(nw1a_d, [P, L * P], dt.bfloat16, "nw1a")
            nw2sb = cload(nw2_d, [P, L * P], dt.bfloat16, "nw2")
            nb1csb = cload(nb1c_d, [P, L], dt.float32, "nb1c")
            nb2bcsb = cload(nb2bc_d, [P, L * P], dt.float32, "nb2bc")
            lngbcsb = cload(lngbc_d, [P, L * P], dt.float32, "lngbc")
            lnbbcsb = cload(lnbbc_d, [P, L * P], dt.float32, "lnbbc")


            def emit_silu(out_ap, in_ap, scratch_pool, tag):
                if not SILU_DECOMPOSED:
                    nc.scalar.activation(out_ap, in_ap, AF.Silu)
                else:
                    sg = scratch_pool.tile(
                        [P, in_ap.shape[-1] if in_ap.ndim == 2 else P],
                        dt.float32, tag=tag)
                    sga = sg[:in_ap.shape[0], :in_ap.shape[-1]]
                    nc.scalar.activation(sga, in_ap, AF.Sigmoid)
                    nc.vector.tensor_tensor(out=out_ap, in0=in_ap, in1=sga,
                                            op=ALU.mult)

            def edge_phase(l, table):
                lsl = slice(l * P, (l + 1) * P)
                for b in range(nblk):
                    gd = gpool.tile([P, BLK_E], dt.bfloat16, tag="gd")
                    gs = gpool.tile([P, BLK_E], dt.bfloat16, tag="gs")
                    for k in range(4):
                        nc.gpsimd.indirect_dma_start(
                            out=gd[:, k * P:(k + 1) * P],
                            out_offset=None,
                            in_=table[:, :],
                            in_offset=IOA(ap=dstidxsb[:, 4 * b + k:4 * b + k + 1],
                                          axis=0))
                        nc.gpsimd.indirect_dma_start(
                            out=gs[:, k * P:(k + 1) * P],
                            out_offset=None,
                            in_=table[:, :],
                            in_offset=IOA(ap=srcidxsb[:, 4 * b + k:4 * b + k + 1],
                                          axis=0))
                    r4 = gpool.tile([4, BLK_E], dt.bfloat16, tag="r4")
                    nc.sync.dma_start(out=r4[:], in_=rel4_d[b])

                    # m1 = H1d[dst] + H1s[src] + [relpos,1] @ [W1c;b1]
                    m1p = pm1.tile([P, BLK_E], dt.float32, tag="m1")
                    nc.tensor.matmul(m1p[:], lhsT=identsb[:], rhs=gd[:],
                                     start=True, stop=False, skip_group_check=True)
                    nc.tensor.matmul(m1p[:], lhsT=identsb[:], rhs=gs[:],
                                     start=False, stop=False, skip_group_check=True)
                    for k in range(4):
                        nc.tensor.matmul(
                            m1p[:, k * P:(k + 1) * P],
                            lhsT=r4[:, k * P:(k + 1) * P],
                            rhs=w1c4sb[:, lsl],
                            start=False, stop=(k == 3), skip_group_check=True)

                    m1s = wpool.tile([P, BLK_E], dt.bfloat16, tag="m1s")
                    emit_silu(m1s[:], m1p[:], wpool, "sg1")

                    # transpose to feature-major for the second edge matmul
                    m1tp = pm1t.tile([P, BLK_E], dt.bfloat16, tag="m1t")
                    for k in range(4):
                        nc.tensor.transpose(
                            m1tp[:, k * P:(k + 1) * P],
                            m1s[:, k * P:(k + 1) * P], identsb[:])
                    m1t = wpool.tile([P, BLK_E], dt.bfloat16, tag="m1tsb")
                    nc.scalar.activation(m1t[:], m1tp[:], AF.Copy)

                    m2p = pm2.tile([P, BLK_E], dt.float32, tag="m2")
                    for k in range(4):
                        nc.tensor.matmul(
                            m2p[:, k * P:(k + 1) * P],
                            lhsT=m1t[:, k * P:(k + 1) * P],
                            rhs=w2sb[:, lsl], start=True, stop=True)
                    nc.vector.tensor_tensor(
                        out=m2p[:].rearrange("p (j h) -> p j h", h=P),
                        in0=m2p[:].rearrange("p (j h) -> p j h", h=P),
                        in1=b2bcsb[:, lsl].unsqueeze(1).to_broadcast([P, 4, P]),
                        op=ALU.add)
                    m2s = wpool.tile([P, BLK_E], dt.bfloat16, tag="m2s")
                    emit_silu(m2s[:], m2p[:], wpool, "sg2")

                    # segment indicator S[e, s] = (dstrel[e] == s)
                    S = wpool.tile([P, BLK_E], dt.bfloat16, tag="S")
                    nc.vector.tensor_tensor(
                        out=S[:].rearrange("p (j s) -> p j s", s=P),
                        in0=dstrelsb[:, 4 * b:4 * b + 4].unsqueeze(2)
                            .to_broadcast([P, 4, P]),
                        in1=iotasb[:].unsqueeze(1).to_broadcast([P, 4, P]),
                        op=ALU.is_equal)

                    aggp = pagg.tile([P, P], dt.float32, tag="agg")
                    for k in range(4):
                        nc.tensor.matmul(
                            aggp[:], lhsT=S[:, k * P:(k + 1) * P],
                            rhs=m2s[:, k * P:(k + 1) * P],
                            start=(k == 0), stop=(k == 3))
                    aggsb = spool.tile([P, P], dt.bfloat16, tag="aggsb")
                    nc.scalar.activation(aggsb[:], aggp[:], AF.Copy)
                    nc.gpsimd.indirect_dma_start(
                        out=agg_hbm[:, :],
                        out_offset=IOA(ap=slotsb[:, b:b + 1], axis=0),
                        in_=aggsb[:], in_offset=None)

            xbuf = cpool.tile([P, NW * P], dt.float32, tag="xbuf")
            varbuf = cpool.tile([P, NW], dt.float32, tag="varbuf")
            sqsb = cpool.tile([P, NW], dt.float32, tag="sqsb")
            rstdb = cpool.tile([P, NW], dt.float32, tag="rstdb")
            nc.vector.memset(varbuf[:], 1.0)

            def node_phase(l, hprev, hnext, h1own_l):
                lsl = slice(l * P, (l + 1) * P)
                last = l == L - 1
                hw_tiles = []
                for w in range(NW):
                    cnt = min(P, nv - w * P)
                    rows = slice(w * P, w * P + cnt)
                    hwin = spool.tile([P, P], dt.bfloat16, tag=f"hwin{w % 2}")
                    awin = spool.tile([P, P], dt.bfloat16, tag="awin")
                    nc.sync.dma_start(out=hwin[:cnt, :], in_=hprev[rows, :])
                    nc.sync.dma_start(out=awin[:cnt, :], in_=agg_hbm[rows, :])
                    hw_tiles.append(hwin)

                    hwtp = pm1t.tile([P, P], dt.bfloat16, tag="m1t")
                    nc.tensor.transpose(hwtp[:, :cnt], hwin[:cnt, :],
                                        identsb[:cnt, :cnt])
                    hwt = spool.tile([P, P], dt.bfloat16, tag="hwt")
                    nc.scalar.activation(hwt[:, :cnt], hwtp[:, :cnt], AF.Copy)
                    awtp = pm1t.tile([P, P], dt.bfloat16, tag="m1t")
                    nc.tensor.transpose(awtp[:, :cnt], awin[:cnt, :],
                                        identsb[:cnt, :cnt])
                    awt = spool.tile([P, P], dt.bfloat16, tag="awt")
                    nc.scalar.activation(awt[:, :cnt], awtp[:, :cnt], AF.Copy)

                    n1p = pm2.tile([P, P], dt.float32, tag="m2")
                    nc.tensor.matmul(n1p[:, :cnt], lhsT=nw1hsb[:, lsl],
                                     rhs=hwt[:, :cnt], start=True, stop=False)
                    nc.tensor.matmul(n1p[:, :cnt], lhsT=nw1asb[:, lsl],
                                     rhs=awt[:, :cnt], start=False, stop=True)
                    nc.vector.tensor_scalar_add(n1p[:, :cnt], n1p[:, :cnt],
                                                nb1csb[:, l:l + 1])
                    n1s = spool.tile([P, P], dt.bfloat16, tag="n1s")
                    emit_silu(n1s[:, :cnt], n1p[:, :cnt], spool, "sgn")

                    n2p = pagg.tile([P, P], dt.float32, tag="agg")
                    nc.tensor.matmul(n2p[:cnt, :], lhsT=n1s[:, :cnt],
                                     rhs=nw2sb[:, lsl], start=True, stop=True)
                    # x = h + node_mlp(h, agg), then center
                    nc.vector.tensor_tensor(out=n2p[:cnt, :], in0=n2p[:cnt, :],
                                            in1=nb2bcsb[:cnt, lsl], op=ALU.add)
                    nc.vector.tensor_tensor(out=n2p[:cnt, :], in0=n2p[:cnt, :],
                                            in1=hwin[:cnt, :], op=ALU.add)
                    mu = spool.tile([P, 1], dt.float32, tag="mu")
                    nc.vector.reduce_sum(mu[:cnt], n2p[:cnt, :],
                                         axis=mybir.AxisListType.X)
                    nc.vector.tensor_scalar_mul(mu[:cnt], mu[:cnt], 1.0 / P)
                    nc.vector.tensor_scalar_sub(n2p[:cnt, :], n2p[:cnt, :],
                                                mu[:cnt])
                    sq = spool.tile([P, P], dt.float32, tag="sq")
                    nc.scalar.activation(sq[:cnt, :], n2p[:cnt, :], AF.Square,
                                         accum_out=varbuf[:cnt, w:w + 1])
                    nc.vector.tensor_copy(xbuf[:cnt, w * P:w * P + P],
                                          n2p[:cnt, :])

                # batched LayerNorm scale: one table-switching Sqrt per layer
                nc.vector.tensor_scalar(varbuf[:], varbuf[:], 1.0 / P, EPS,
                                        op0=ALU.mult, op1=ALU.add)
                nc.scalar.activation(sqsb[:], varbuf[:], AF.Sqrt)
                nc.vector.reciprocal(rstdb[:], sqsb[:])

                for w in range(NW):
                    cnt = min(P, nv - w * P)
                    rows = slice(w * P, w * P + cnt)
                    xsl = slice(w * P, w * P + P)
                    xn = spool.tile([P, P], dt.float32, tag="xn")
                    nc.vector.tensor_scalar_mul(xn[:cnt, :], xbuf[:cnt, xsl],
                                                rstdb[:cnt, w:w + 1])
                    nc.vector.tensor_tensor(out=xn[:cnt, :], in0=xn[:cnt, :],
                                            in1=lngbcsb[:cnt, lsl], op=ALU.mult)
                    nc.vector.tensor_tensor(out=xn[:cnt, :], in0=xn[:cnt, :],
                                            in1=lnbbcsb[:cnt, lsl], op=ALU.add)
                    if last:
                        nc.sync.dma_start(out=hout_d[rows, :], in_=xn[:cnt, :])
                    else:
                        hnb = spool.tile([P, P], dt.bfloat16, tag="hnb")
                        nc.vector.tensor_copy(hnb[:cnt, :], xn[:cnt, :])
                        nc.sync.dma_start(out=hnext[rows, :], in_=hnb[:cnt, :])
                        # next layer's projected gather tables
                        hntp = pm1t.tile([P, P], dt.bfloat16, tag="m1t")
                        nc.tensor.transpose(hntp[:, :cnt], hnb[:cnt, :],
                                            identsb[:cnt, :cnt])
                        hnt = spool.tile([P, P], dt.bfloat16, tag="hnt")
                        nc.scalar.activation(hnt[:, :cnt], hntp[:, :cnt], AF.Copy)
                        nsl = slice((l + 1) * P, (l + 2) * P)
                        pd = pm2.tile([P, P], dt.float32, tag="m2")
                        nc.tensor.matmul(pd[:cnt, :], lhsT=hnt[:, :cnt],
                                         rhs=w1asb[:, nsl], start=True, stop=True)
                        pdb = spool.tile([P, P], dt.bfloat16, tag="pdb")
                        nc.scalar.activation(pdb[:cnt, :], pd[:cnt, :], AF.Copy)
                        nc.sync.dma_start(out=h1own_l[rows, :], in_=pdb[:cnt, :])
                        ps = pm1.tile([P, P], dt.float32, tag="m1")
                        nc.tensor.matmul(ps[:cnt, :], lhsT=hnt[:, :cnt],
                                         rhs=w1bsb[:, nsl], start=True, stop=True)
                        psb = spool.tile([P, P], dt.bfloat16, tag="psb")
                        nc.scalar.activation(psb[:cnt, :], ps[:cnt, :], AF.Copy)
                        nc.sync.dma_start(
                            out=h1own_l[w * P + nv: w * P + nv + cnt, :],
                            in_=psb[:cnt, :])

            for l in range(L):
                table = h1f0 if l == 0 else h1full[l - 1]
                hprev = h0own if l == 0 else hown[l - 1]
                edge_phase(l, table)
                if l < L - 1:
                    node_phase(l, hprev, hown[l], h1own[l])
                    nc.gpsimd.collective_compute(
                        "AllGather", mybir.AluOpType.bypass,
                        replica_groups=groups,
                        ins=[h1own[l][:, :]], outs=[h1full[l][:, :]])
                else:
                    node_phase(l, hprev, None, None)

    nc.finalize()
    return nc


# ----------------------------------------------------------------------------
# Entry point
# ----------------------------------------------------------------------------

def _make_in_maps(geom, per_core, weights, host):
    in_maps = []
    nv = geom["nv"]
    for c in range(NCORES):
        pc = per_core[c]
        m = {
            "h1full0": host["h1full0"],
            "h0own": host["h0"][c * nv:(c + 1) * nv].astype(BF16),
            "dstidx": pc["dstidx"], "srcidx": pc["srcidx"],
            "dstrel": pc["dstrel"], "slots": pc["slots"],
            "rel4": pc["rel4"],
        }
        m.update(weights)
        in_maps.append(m)
    return in_maps


def _postprocess(host, houts):
    h = np.concatenate(houts, axis=0).astype(np.float32)
    v_pred = h @ host["op_w"] + host["op_b"]
    diff = v_pred - host["target"]
    return np.float32(np.mean(diff.astype(np.float64) ** 2))


def kernel(**inputs):
    from concourse.bass_utils import run_bass_kernel_spmd

    geom, per_core, weights, host = _preprocess(inputs)
    nc = _build_program(geom)
    in_maps = _make_in_maps(geom, per_core, weights, host)
    res = run_bass_kernel_spmd(nc, in_maps, list(range(NCORES)))
    houts = [res.results[c]["hout"] for c in range(NCORES)]
    return _postprocess(host, houts)



# revision 8
# speedup vs baseline: 1.7479x; 1.7479x over previous
"""Trainium2 Bass kernel for an MLP flow-matching GNN (message passing).

Strategy (8 NeuronCores, SPMD):
  - Host: sort edges by destination, partition nodes into 8 contiguous ranges
    (one per core), pad each core's node count to a multiple of 128.  Nodes are
    processed in UNIFORM groups of 128; each group's incident edges are padded
    to a uniform per-group edge capacity (EG = PPG*128, PPG = max group degree
    sum over all cores, in panels of 128).  This makes every slice bound
    core-uniform so a single SPMD program works for all cores.
  - The edge-MLP first layer is pre-projected into per-node tables:
        h1d = h @ W1a + x_t @ W1c + b1      (dst table, local per core)
        h1s = h @ W1b - x_t @ W1c           (src table, AllGathered)
    so that m1[e] = h1d[dst] + h1s[src] includes the relative-position term
    (rel @ W1c = Y[dst] - Y[src]) and bias with no per-edge matmul.
  - Device, per layer:
      edge phase:  two batched indirect-DMA gathers per chunk of groups (dst
                   rows, then src rows accumulated via the DMA CCE add) ->
                   SiLU -> PE transpose -> edge-MLP2 matmul -> +b2, SiLU ->
                   segment-sum via matmul against an is_equal indicator,
                   accumulated in PSUM across the whole 128-node group ->
                   one contiguous store per group.
      node phase:  DMA-transpose loads of h and agg (feature-major), node MLP
                   + residual + chunk-local LayerNorm, and the next layer's
                   projected tables (with the x_t @ W1c fold done by a K=4
                   accumulate matmul).
      comm:        one AllGather of the src table only.
  - Host: final projection + MSE loss (tiny).
"""

import numpy as np
import ml_dtypes

BF16 = ml_dtypes.bfloat16
EPS = 1e-5
NCORES = 8
P = 128          # partition width / hidden size (H must equal 128)
GCH = 7          # groups gathered per indirect-DMA instruction pair
CW = 4           # node windows per node-phase chunk


def _silu(x):
    return x * (1.0 / (1.0 + np.exp(-x)))


# ----------------------------------------------------------------------------
# Host-side preprocessing
# ----------------------------------------------------------------------------

def _preprocess(inputs):
    pos0 = np.asarray(inputs["pos0"], np.float32)
    pos1 = np.asarray(inputs["pos1"], np.float32)
    z = np.asarray(inputs["z"], np.float32)
    t = np.asarray(inputs["t"], np.float32)
    edge_index = np.asarray(inputs["edge_index"])
    batch = np.asarray(inputs["batch"])
    ew1 = np.asarray(inputs["ew1"], np.float32)
    eb1 = np.asarray(inputs["eb1"], np.float32)
    ew2 = np.asarray(inputs["ew2"], np.float32)
    eb2 = np.asarray(inputs["eb2"], np.float32)
    nw1 = np.asarray(inputs["nw1"], np.float32)
    nb1 = np.asarray(inputs["nb1"], np.float32)
    nw2 = np.asarray(inputs["nw2"], np.float32)
    nb2 = np.asarray(inputs["nb2"], np.float32)
    ln_g = np.asarray(inputs["ln_g"], np.float32)
    ln_b = np.asarray(inputs["ln_b"], np.float32)

    V = pos0.shape[0]
    L = ew1.shape[0]
    H = ew1.shape[2]
    assert H == P
    nv = V // NCORES
    assert nv * NCORES == V
    ngrp = (nv + P - 1) // P
    nvp = ngrp * P

    ts = float(t[0])
    x_t = (1.0 - ts) * pos0 + ts * pos1
    target = pos1 - pos0

    te_w1 = np.asarray(inputs["te_w1"], np.float32)
    te_b1 = np.asarray(inputs["te_b1"], np.float32)
    te_w2 = np.asarray(inputs["te_w2"], np.float32)
    te_b2 = np.asarray(inputs["te_b2"], np.float32)
    cp_w = np.asarray(inputs["cp_w"], np.float32)
    cp_b = np.asarray(inputs["cp_b"], np.float32)

    t_emb = _silu(np.array([[ts]], np.float32) @ te_w1 + te_b1) @ te_w2 + te_b2
    h0 = np.concatenate(
        [z[batch], np.broadcast_to(t_emb, (V, t_emb.shape[1]))], axis=1
    ) @ cp_w + cp_b  # [V, H] f32

    # layer-0 folded tables (rel-pos + bias folded in; see module docstring)
    Y = x_t @ ew1[0, 2 * H:2 * H + 3]                 # [V, H]
    h1d0 = h0 @ ew1[0, :H] + Y + eb1[0]
    h1s0 = h0 @ ew1[0, H:2 * H] - Y

    def pad_rows(a):
        out = np.zeros((nvp, a.shape[1]), a.dtype)
        out[:a.shape[0]] = a
        return out

    h1s_full0 = np.concatenate(
        [pad_rows(h1s0[c * nv:(c + 1) * nv]) for c in range(NCORES)], axis=0
    ).astype(BF16)                                    # [8*nvp, H]

    # edges sorted by destination, split at core boundaries
    src_g = edge_index[0].astype(np.int64)
    dst_g = edge_index[1].astype(np.int64)
    order = np.argsort(dst_g, kind="stable")
    dst_s = dst_g[order]
    src_s = src_g[order]
    bounds = np.searchsorted(dst_s, np.arange(0, V + 1, nv))

    # uniform per-group edge capacity
    maxe = 0
    per_edges = []
    for c in range(NCORES):
        e0, e1 = int(bounds[c]), int(bounds[c + 1])
        ld = (dst_s[e0:e1] - c * nv).astype(np.int64)
        sg = src_s[e0:e1]
        grp = ld // P
        cnt = np.bincount(grp, minlength=ngrp)
        maxe = max(maxe, int(cnt.max()))
        per_edges.append((ld, sg, cnt))
    PPG = (maxe + P - 1) // P
    EG = PPG * P
    ncol = ngrp * PPG

    per_core = []
    for c in range(NCORES):
        ld, sg, cnt = per_edges[c]
        srcidx = np.zeros(ngrp * EG, np.int32)
        dstrel = np.full(ngrp * EG, -1, np.int32)
        starts = np.concatenate([[0], np.cumsum(cnt)])
        for g in range(ngrp):
            s0, s1 = int(starts[g]), int(starts[g + 1])
            n = s1 - s0
            sl = slice(g * EG, g * EG + n)
            srow = (sg[s0:s1] // nv) * nvp + (sg[s0:s1] % nv)
            srcidx[sl] = srow
            dstrel[sl] = ld[s0:s1] - g * P
        # slot (j, p) = edge j*128 + p  ->  column-major [P, ncol]
        per_core.append({
            "srcidx": srcidx.reshape(ncol, P).T.copy(),
            "dstrel": dstrel.reshape(ncol, P).T.copy(),
            "dstrelF": dstrel.astype(BF16)[None, :].copy(),
            "h1d_own0": pad_rows(h1d0[c * nv:(c + 1) * nv]).astype(BF16),
            "h0own": pad_rows(h0[c * nv:(c + 1) * nv]).astype(BF16),
            "x4own": np.concatenate(
                [pad_rows(x_t[c * nv:(c + 1) * nv]).T,
                 np.ones((1, nvp), np.float32)], axis=0),   # [4, nvp]
        })

    # device weight layouts (concat layers along free dim)
    cat = lambda m: np.concatenate([m[l] for l in range(L)], axis=1)
    bc = lambda v: np.concatenate(
        [np.broadcast_to(v[l], (P, H)) for l in range(L)], axis=1)
    w1a = cat(ew1[:, :H]).astype(BF16)
    w1b = cat(ew1[:, H:2 * H]).astype(BF16)
    w2 = cat(ew2).astype(BF16)
    w1c4d = np.concatenate(
        [np.concatenate([ew1[l, 2 * H:], eb1[l][None, :]], 0)
         for l in range(L)], axis=1).astype(np.float32)        # [4, L*H]
    w1c4s = np.concatenate(
        [np.concatenate([-ew1[l, 2 * H:], np.zeros((1, H), np.float32)], 0)
         for l in range(L)], axis=1).astype(np.float32)
    weights = dict(
        w1a=w1a, w1b=w1b, w2=w2, w1c4d=w1c4d, w1c4s=w1c4s,
        b2bc=bc(eb2).astype(np.float32),
        nw1h=cat(nw1[:, :H]).astype(BF16),
        nw1a=cat(nw1[:, H:]).astype(BF16),
        nw2=cat(nw2).astype(BF16),
        nb1c=nb1.T.astype(np.float32).copy(),                  # [H, L]
        nb2bc=bc(nb2).astype(np.float32),
        lngbc=bc(ln_g).astype(np.float32),
        lnbbc=bc(ln_b).astype(np.float32),
        ident=np.eye(P, dtype=BF16),
        iota=np.tile(np.arange(P, dtype=np.int32), (P, 1)),
        iotacolf=np.arange(P, dtype=np.float32)[:, None].copy(),
    )

    geom = dict(V=V, L=L, H=H, nv=nv, nvp=nvp, ngrp=ngrp, PPG=PPG)
    host = dict(h1s_full0=h1s_full0, target=target,
                op_w=np.asarray(inputs["op_w"], np.float32),
                op_b=np.asarray(inputs["op_b"], np.float32))
    return geom, per_core, weights, host


# ----------------------------------------------------------------------------
# Device program
# ----------------------------------------------------------------------------

SILU_DECOMPOSED = False  # sim has no Silu table; set True for CoreSim runs


def _build_program(geom):
    import concourse.bass as bass
    import concourse.bacc as bacc
    import concourse.mybir as mybir
    import concourse.tile as tile

    dt = mybir.dt
    AF = mybir.ActivationFunctionType
    ALU = mybir.AluOpType
    IOA = bass.IndirectOffsetOnAxis
    AX = mybir.AxisListType

    L, nvp, ngrp, PPG = geom["L"], geom["nvp"], geom["ngrp"], geom["PPG"]
    ncol = ngrp * PPG
    NW = ngrp  # node windows per core

    nc = bacc.Bacc(num_devices=NCORES)

    # ---- I/O ----
    h1sf0 = nc.declare_dram_parameter("h1s_full0", [NCORES * nvp, P],
                                      dt.bfloat16, isOutput=False)
    h1d0_d = nc.declare_dram_parameter("h1d_own0", [nvp, P], dt.bfloat16,
                                       isOutput=False)
    h0own = nc.declare_dram_parameter("h0own", [nvp, P], dt.bfloat16,
                                      isOutput=False)
    srcidx_d = nc.declare_dram_parameter("srcidx", [P, ncol], dt.int32, isOutput=False)
    dstrel_d = nc.declare_dram_parameter("dstrel", [P, ncol], dt.int32, isOutput=False)
    dstrelF_d = nc.declare_dram_parameter("dstrelF", [1, ncol * P], dt.bfloat16, isOutput=False)
    x4own_d = nc.declare_dram_parameter("x4own", [4, nvp], dt.float32, isOutput=False)
    w1a_d = nc.declare_dram_parameter("w1a", [P, L * P], dt.bfloat16, isOutput=False)
    w1b_d = nc.declare_dram_parameter("w1b", [P, L * P], dt.bfloat16, isOutput=False)
    w2_d = nc.declare_dram_parameter("w2", [P, L * P], dt.bfloat16, isOutput=False)
    w1c4d_d = nc.declare_dram_parameter("w1c4d", [4, L * P], dt.float32, isOutput=False)
    w1c4s_d = nc.declare_dram_parameter("w1c4s", [4, L * P], dt.float32, isOutput=False)
    b2bc_d = nc.declare_dram_parameter("b2bc", [P, L * P], dt.float32, isOutput=False)
    nw1h_d = nc.declare_dram_parameter("nw1h", [P, L * P], dt.bfloat16, isOutput=False)
    nw1a_d = nc.declare_dram_parameter("nw1a", [P, L * P], dt.bfloat16, isOutput=False)
    nw2_d = nc.declare_dram_parameter("nw2", [P, L * P], dt.bfloat16, isOutput=False)
    nb1c_d = nc.declare_dram_parameter("nb1c", [P, L], dt.float32, isOutput=False)
    nb2bc_d = nc.declare_dram_parameter("nb2bc", [P, L * P], dt.float32, isOutput=False)
    lngbc_d = nc.declare_dram_parameter("lngbc", [P, L * P], dt.float32, isOutput=False)
    lnbbc_d = nc.declare_dram_parameter("lnbbc", [P, L * P], dt.float32, isOutput=False)
    ident_d = nc.declare_dram_parameter("ident", [P, P], dt.bfloat16, isOutput=False)
    iota_d = nc.declare_dram_parameter("iota", [P, P], dt.int32, isOutput=False)
    iotacolf_d = nc.declare_dram_parameter("iotacolf", [P, 1], dt.float32, isOutput=False)
    hout_d = nc.declare_dram_parameter("hout", [nvp, P], dt.float32, isOutput=True)

    # ---- internal DRAM ----
    agg_hbm = nc.dram_tensor("agg_hbm", [nvp, P], dt.bfloat16)
    hown = [nc.dram_tensor(f"hown{l + 1}", [nvp, P], dt.bfloat16)
            for l in range(L - 1)]
    h1down = [nc.dram_tensor(f"h1down{l + 1}", [nvp, P], dt.bfloat16)
              for l in range(L - 1)]
    h1sown = [nc.dram_tensor(f"h1sown{l + 1}", [nvp, P], dt.bfloat16)
              for l in range(L - 1)]
    h1sfull = [nc.dram_tensor(f"h1sfull{l + 1}", [NCORES * nvp, P],
                              dt.bfloat16, addr_space="Shared")
               for l in range(L - 1)]

    groups = [list(range(NCORES))]

    with tile.TileContext(nc) as tc:
        with (
            tc.tile_pool(name="const", bufs=1) as cpool,
            tc.tile_pool(name="gather", bufs=2) as gpool,
            tc.tile_pool(name="work", bufs=3) as wpool,
            tc.tile_pool(name="small", bufs=4) as spool,
            tc.tile_pool(name="pt", bufs=2, space="PSUM") as pt,
            tc.tile_pool(name="pmB", bufs=4, space="PSUM") as pmB,
            tc.tile_pool(name="pagg", bufs=2, space="PSUM") as pagg,
        ):
            def cload(src, shape, dtype, tag):
                t_ = cpool.tile(shape, dtype, tag=tag)
                nc.sync.dma_start(out=t_[:], in_=src[:, :])
                return t_

            identsb = cload(ident_d, [P, P], dt.bfloat16, "ident")
            iotasb = cload(iota_d, [P, P], dt.int32, "iota")
            onesb = cpool.tile([1, P], dt.bfloat16, tag="ones")
            nc.vector.memset(onesb[:], 1.0)
            iotacol = cload(iotacolf_d, [P, 1], dt.float32, "iotacol")
            srcidxsb = cload(srcidx_d, [P, ncol], dt.int32, "srcidx")
            dstrelsb = cload(dstrel_d, [P, ncol], dt.int32, "dstrel")
            x4sb = cload(x4own_d, [4, nvp], dt.float32, "x4own")
            w1asb = cload(w1a_d, [P, L * P], dt.bfloat16, "w1a")
            w1bsb = cload(w1b_d, [P, L * P], dt.bfloat16, "w1b")
            w2sb = cload(w2_d, [P, L * P], dt.bfloat16, "w2")
            w1c4dsb = cload(w1c4d_d, [4, L * P], dt.float32, "w1c4d")
            w1c4ssb = cload(w1c4s_d, [4, L * P], dt.float32, "w1c4s")
            b2bcsb = cload(b2bc_d, [P, L * P], dt.float32, "b2bc")
            nw1hsb = cload(nw1h_d, [P, L * P], dt.bfloat16, "nw1h")
            nw1asb = cload(nw1a_d, [P, L * P], dt.bfloat16, "nw1a")
            nw2sb = cload(nw2_d, [P, L * P], dt.bfloat16, "nw2")
            nb1csb = cload(nb1c_d, [P, L], dt.float32, "nb1c")
            nb2bcsb = cload(nb2bc_d, [P, L * P], dt.float32, "nb2bc")
            lngbcsb = cload(lngbc_d, [P, L * P], dt.float32, "lngbc")
            lnbbcsb = cload(lnbbc_d, [P, L * P], dt.float32, "lnbbc")

            def emit_silu(out_ap, in_ap, scratch_pool, tag, bias=0.0):
                if not SILU_DECOMPOSED:
                    nc.scalar.activation(out_ap, in_ap, AF.Silu, bias=bias)
                else:
                    sg = scratch_pool.tile(
                        [P, in_ap.shape[-1] if in_ap.ndim == 2 else P],
                        dt.float32, tag=tag)
                    sga = sg[:in_ap.shape[0], :in_ap.shape[-1]]
                    nc.scalar.activation(sga, in_ap, AF.Sigmoid, bias=bias)
                    nc.vector.tensor_tensor(out=out_ap, in0=in_ap, in1=sga,
                                            op=ALU.mult)

            def edge_phase(l, h1d_dram, h1s_dram):
                lsl = slice(l * P, (l + 1) * P)
                for g0 in range(0, ngrp, GCH):
                    g1 = min(g0 + GCH, ngrp)
                    nrows = (g1 - g0) * P
                    # dst node rows for these groups (contiguous, node-major)
                    nodes = gpool.tile([P, GCH * P], dt.bfloat16, tag="nodes")
                    nc.sync.dma_start(
                        out=nodes[:, :nrows].rearrange("p (g h) -> p g h", h=P),
                        in_=h1d_dram[g0 * P:g1 * P, :].rearrange(
                            "(g p) h -> p g h", p=P))
                    # per-edge slot ids along the free axis (for S_T build)
                    relf = gpool.tile([1, GCH * PPG * P], dt.bfloat16, tag="relf")
                    nc.sync.dma_start(
                        out=relf[:, :(g1 - g0) * PPG * P],
                        in_=dstrelF_d[:, g0 * PPG * P:g1 * PPG * P])
                    for g in range(g0, g1):
                        aggp = pagg.tile([P, P], dt.float32, tag="agg")
                        nodes_g = nodes[:, (g - g0) * P:(g - g0 + 1) * P]
                        for js in range(0, PPG, 4):
                            je = min(js + 4, PPG)
                            nsb = je - js
                            # src rows: one indirect gather per 128-edge panel
                            gs = wpool.tile([P, 4 * P], dt.bfloat16, tag="gs")
                            for k in range(nsb):
                                col = g * PPG + js + k
                                nc.gpsimd.indirect_dma_start(
                                    out=gs[:, k * P:(k + 1) * P],
                                    out_offset=None,
                                    in_=h1s_dram[:, :],
                                    in_offset=IOA(ap=srcidxsb[:, col:col + 1],
                                                  axis=0))
                            # S_T[slot, e] via ones-matmul broadcast + is_equal
                            rsl = slice(((g - g0) * PPG + js) * P,
                                        ((g - g0) * PPG + je) * P)
                            bcp = pmB.tile([P, 4 * P], dt.float32, tag="B")
                            nc.tensor.matmul(bcp[:, :nsb * P], lhsT=onesb[:],
                                             rhs=relf[:, rsl],
                                             start=True, stop=True)
                            ST = wpool.tile([P, 4 * P], dt.bfloat16, tag="ST")
                            nc.vector.tensor_scalar(
                                ST[:, :nsb * P], bcp[:, :nsb * P],
                                iotacol[:], None, op0=ALU.is_equal)
                            # m1 = h1s[src] + h1d-expansion  (edge-major, PSUM)
                            m1p = pmB.tile([P, 4 * P], dt.float32, tag="B")
                            nc.tensor.matmul(m1p[:, :nsb * P], lhsT=identsb[:],
                                             rhs=gs[:, :nsb * P],
                                             start=True, stop=False,
                                             skip_group_check=True)
                            for k in range(nsb):
                                nc.tensor.matmul(
                                    m1p[:, k * P:(k + 1) * P],
                                    lhsT=ST[:, k * P:(k + 1) * P],
                                    rhs=nodes_g,
                                    start=False, stop=(k == nsb - 1),
                                    skip_group_check=True)
                            m1s = wpool.tile([P, 4 * P], dt.bfloat16, tag="m1s")
                            emit_silu(m1s[:, :nsb * P], m1p[:, :nsb * P],
                                      wpool, "sg1")
                            m1tp = pt.tile([P, 4 * P], dt.bfloat16, tag="m1t")
                            for k in range(nsb):
                                nc.tensor.transpose(
                                    m1tp[:, k * P:(k + 1) * P],
                                    m1s[:, k * P:(k + 1) * P], identsb[:])
                            m1t = wpool.tile([P, 4 * P], dt.bfloat16, tag="m1tsb")
                            nc.scalar.activation(m1t[:, :nsb * P],
                                                 m1tp[:, :nsb * P], AF.Copy)
                            m2p = pmB.tile([P, 4 * P], dt.float32, tag="B")
                            for k in range(nsb):
                                nc.tensor.matmul(
                                    m2p[:, k * P:(k + 1) * P],
                                    lhsT=m1t[:, k * P:(k + 1) * P],
                                    rhs=w2sb[:, lsl], start=True, stop=True)
                            nc.vector.tensor_tensor(
                                out=m2p[:, :nsb * P].rearrange(
                                    "p (j h) -> p j h", h=P),
                                in0=m2p[:, :nsb * P].rearrange(
                                    "p (j h) -> p j h", h=P),
                                in1=b2bcsb[:, lsl].unsqueeze(1)
                                    .to_broadcast([P, nsb, P]),
                                op=ALU.add)
                            m2s = wpool.tile([P, 4 * P], dt.bfloat16, tag="m2s")
                            emit_silu(m2s[:, :nsb * P], m2p[:, :nsb * P],
                                      wpool, "sg2")
                            S = wpool.tile([P, 4 * P], dt.bfloat16, tag="S")
                            csl = slice(g * PPG + js, g * PPG + je)
                            nc.vector.tensor_tensor(
                                out=S[:, :nsb * P].rearrange(
                                    "p (j s) -> p j s", s=P),
                                in0=dstrelsb[:, csl].unsqueeze(2)
                                    .to_broadcast([P, nsb, P]),
                                in1=iotasb[:].unsqueeze(1)
                                    .to_broadcast([P, nsb, P]),
                                op=ALU.is_equal)
                            for k in range(nsb):
                                nc.tensor.matmul(
                                    aggp[:], lhsT=S[:, k * P:(k + 1) * P],
                                    rhs=m2s[:, k * P:(k + 1) * P],
                                    start=(js == 0 and k == 0),
                                    stop=(je == PPG and k == nsb - 1),
                                    skip_group_check=True)
                        aggsb = spool.tile([P, P], dt.bfloat16, tag="aggsb")
                        nc.scalar.activation(aggsb[:], aggp[:], AF.Copy)
                        nc.sync.dma_start(out=agg_hbm[g * P:(g + 1) * P, :],
                                          in_=aggsb[:])

            def node_phase(l, hprev, hnext, h1d_next, h1s_next):
                lsl = slice(l * P, (l + 1) * P)
                last = l == L - 1
                for w0 in range(0, NW, CW):
                    w1 = min(w0 + CW, NW)
                    nw = w1 - w0
                    cnt = nw * P
                    rows = slice(w0 * P, w0 * P + cnt)
                    hwt = spool.tile([P, CW * P], dt.bfloat16, tag="hwt")
                    nc.sync.dma_start_transpose(hwt[:, :cnt], hprev[rows, :])
                    awt = spool.tile([P, CW * P], dt.bfloat16, tag="awt")
                    nc.sync.dma_start_transpose(awt[:, :cnt], agg_hbm[rows, :])
                    hwin = spool.tile([P, CW * P], dt.bfloat16, tag="hwin")
                    nc.sync.dma_start(
                        out=hwin[:, :cnt].rearrange("p (w h) -> p w h", h=P),
                        in_=hprev[rows, :].rearrange("(w p) h -> p w h", p=P))

                    n1p = pmB.tile([P, CW * P], dt.float32, tag="B")
                    nc.tensor.matmul(n1p[:, :cnt], lhsT=nw1hsb[:, lsl],
                                     rhs=hwt[:, :cnt], start=True, stop=False)
                    nc.tensor.matmul(n1p[:, :cnt], lhsT=nw1asb[:, lsl],
                                     rhs=awt[:, :cnt], start=False, stop=True)
                    n1s = spool.tile([P, CW * P], dt.bfloat16, tag="n1s")
                    emit_silu(n1s[:, :cnt], n1p[:, :cnt], spool, "sgn",
                              bias=nb1csb[:, l:l + 1])

                    n2p = pmB.tile([P, CW * P], dt.float32, tag="B")
                    for w in range(nw):
                        nc.tensor.matmul(n2p[:, w * P:(w + 1) * P],
                                         lhsT=n1s[:, w * P:(w + 1) * P],
                                         rhs=nw2sb[:, lsl],
                                         start=True, stop=True)
                    nc.vector.tensor_tensor(
                        out=n2p[:, :cnt].rearrange("p (w h) -> p w h", h=P),
                        in0=n2p[:, :cnt].rearrange("p (w h) -> p w h", h=P),
                        in1=nb2bcsb[:, lsl].unsqueeze(1)
                            .to_broadcast([P, nw, P]),
                        op=ALU.add)
                    nc.vector.tensor_tensor(
                        out=n2p[:, :cnt].rearrange("p (w h) -> p w h", h=P),
                        in0=n2p[:, :cnt].rearrange("p (w h) -> p w h", h=P),
                        in1=hwin[:, :cnt].rearrange("p (w h) -> p w h", h=P),
                        op=ALU.add)

                    mu = spool.tile([P, CW], dt.float32, tag="mu")
                    var = spool.tile([P, CW], dt.float32, tag="var")
                    sq = spool.tile([P, P], dt.float32, tag="sq")
                    for w in range(nw):
                        wsl = slice(w * P, (w + 1) * P)
                        nc.vector.reduce_sum(mu[:, w:w + 1], n2p[:, wsl],
                                             axis=AX.X)
                    nc.vector.tensor_scalar_mul(mu[:, :nw], mu[:, :nw], 1.0 / P)
                    for w in range(nw):
                        wsl = slice(w * P, (w + 1) * P)
                        nc.vector.tensor_scalar_sub(n2p[:, wsl], n2p[:, wsl],
                                                    mu[:, w:w + 1])
                        nc.scalar.activation(sq[:], n2p[:, wsl], AF.Square,
                                             accum_out=var[:, w:w + 1])
                    nc.vector.tensor_scalar(var[:, :nw], var[:, :nw],
                                            1.0 / P, EPS,
                                            op0=ALU.mult, op1=ALU.add)
                    srt = spool.tile([P, CW], dt.float32, tag="srt")
                    nc.scalar.activation(srt[:, :nw], var[:, :nw], AF.Sqrt)
                    rstd = spool.tile([P, CW], dt.float32, tag="rstd")
                    nc.vector.reciprocal(rstd[:, :nw], srt[:, :nw])

                    xn = spool.tile([P, CW * P], dt.float32, tag="xn")
                    for w in range(nw):
                        wsl = slice(w * P, (w + 1) * P)
                        nc.vector.tensor_scalar_mul(xn[:, wsl], n2p[:, wsl],
                                                    rstd[:, w:w + 1])
                    nc.vector.tensor_tensor(
                        out=xn[:, :cnt].rearrange("p (w h) -> p w h", h=P),
                        in0=xn[:, :cnt].rearrange("p (w h) -> p w h", h=P),
                        in1=lngbcsb[:, lsl].unsqueeze(1)
                            .to_broadcast([P, nw, P]),
                        op=ALU.mult)
                    nc.vector.tensor_tensor(
                        out=xn[:, :cnt].rearrange("p (w h) -> p w h", h=P),
                        in0=xn[:, :cnt].rearrange("p (w h) -> p w h", h=P),
                        in1=lnbbcsb[:, lsl].unsqueeze(1)
                            .to_broadcast([P, nw, P]),
                        op=ALU.add)

                    if last:
                        nc.sync.dma_start(
                            out=hout_d[rows, :].rearrange(
                                "(w p) h -> p w h", p=P),
                            in_=xn[:, :cnt].rearrange("p (w h) -> p w h", h=P))
                        continue

                    hnb = spool.tile([P, CW * P], dt.bfloat16, tag="hnb")
                    nc.vector.tensor_copy(hnb[:, :cnt], xn[:, :cnt])
                    nc.sync.dma_start(
                        out=hnext[rows, :].rearrange("(w p) h -> p w h", p=P),
                        in_=hnb[:, :cnt].rearrange("p (w h) -> p w h", h=P))
                    hnt = spool.tile([P, CW * P], dt.bfloat16, tag="hnt")
                    nc.sync.dma_start_transpose(hnt[:, :cnt], hnext[rows, :])

                    nsl = slice((l + 1) * P, (l + 2) * P)
                    for (tbl, wab, w1c4b, tag) in (
                        (h1d_next, w1asb, w1c4dsb, "pd"),
                        (h1s_next, w1bsb, w1c4ssb, "ps"),
                    ):
                        pp = pmB.tile([P, CW * P], dt.float32, tag="B")
                        for w in range(nw):
                            wsl = slice(w * P, (w + 1) * P)
                            nc.tensor.matmul(pp[:, wsl],
                                             lhsT=hnt[:, wsl],
                                             rhs=wab[:, nsl],
                                             start=True, stop=False)
                            nc.tensor.matmul(
                                pp[:, wsl],
                                lhsT=x4sb[:, w0 * P + w * P:
                                          w0 * P + (w + 1) * P],
                                rhs=w1c4b[:, nsl],
                                start=False, stop=True)
                        pb = spool.tile([P, CW * P], dt.bfloat16, tag=tag)
                        nc.scalar.activation(pb[:, :cnt], pp[:, :cnt], AF.Copy)
                        nc.sync.dma_start(
                            out=tbl[rows, :].rearrange("(w p) h -> p w h", p=P),
                            in_=pb[:, :cnt].rearrange("p (w h) -> p w h", h=P))

            for l in range(L):
                h1d_dram = h1d0_d if l == 0 else h1down[l - 1]
                h1s_dram = h1sf0 if l == 0 else h1sfull[l - 1]
                hprev = h0own if l == 0 else hown[l - 1]
                edge_phase(l, h1d_dram, h1s_dram)
                if l < L - 1:
                    node_phase(l, hprev, hown[l], h1down[l], h1sown[l])
                    nc.gpsimd.collective_compute(
                        "AllGather", mybir.AluOpType.bypass,
                        replica_groups=groups,
                        ins=[h1sown[l][:, :]], outs=[h1sfull[l][:, :]])
                else:
                    node_phase(l, hprev, None, None, None)

    nc.finalize()
    return nc


# ----------------------------------------------------------------------------
# Entry point
# ----------------------------------------------------------------------------

def _make_in_maps(geom, per_core, weights, host):
    in_maps = []
    for c in range(NCORES):
        pc = per_core[c]
        m = {
            "h1s_full0": host["h1s_full0"],
            "h1d_own0": pc["h1d_own0"],
            "h0own": pc["h0own"],
            "srcidx": pc["srcidx"], "dstrel": pc["dstrel"],
            "dstrelF": pc["dstrelF"], "x4own": pc["x4own"],
        }
        m.update(weights)
        in_maps.append(m)
    return in_maps


def _postprocess(geom, host, houts):
    nv = geom["nv"]
    h = np.concatenate([ho[:nv] for ho in houts], axis=0).astype(np.float32)
    v_pred = h @ host["op_w"] + host["op_b"]
    diff = v_pred - host["target"]
    return np.float32(np.mean(diff.astype(np.float64) ** 2))


def kernel(**inputs):
    from concourse.bass_utils import run_bass_kernel_spmd

    geom, per_core, weights, host = _preprocess(inputs)
    nc = _build_program(geom)
    in_maps = _make_in_maps(geom, per_core, weights, host)
    res = run_bass_kernel_spmd(nc, in_maps, list(range(NCORES)))
    houts = [res.results[c]["hout"] for c in range(NCORES)]
    return _postprocess(geom, host, houts)


# revision 9
# speedup vs baseline: 1.7636x; 1.0090x over previous
"""Trainium2 Bass kernel for an MLP flow-matching GNN (message passing).

Strategy (8 NeuronCores, SPMD):
  - Host: sort edges by destination, partition nodes into 8 contiguous ranges
    (one per core), pad each core's node count to a multiple of 128.  Nodes are
    processed in UNIFORM groups of 128; each group's incident edges are padded
    to a uniform per-group edge capacity (EG = PPG*128, PPG = max group degree
    sum over all cores, in panels of 128).  This makes every slice bound
    core-uniform so a single SPMD program works for all cores.
  - The edge-MLP first layer is pre-projected into per-node tables:
        h1d = h @ W1a + x_t @ W1c + b1      (dst table, local per core)
        h1s = h @ W1b - x_t @ W1c           (src table, AllGathered)
    so that m1[e] = h1d[dst] + h1s[src] includes the relative-position term
    (rel @ W1c = Y[dst] - Y[src]) and bias with no per-edge matmul.
  - Device, per layer:
      edge phase:  two batched indirect-DMA gathers per chunk of groups (dst
                   rows, then src rows accumulated via the DMA CCE add) ->
                   SiLU -> PE transpose -> edge-MLP2 matmul -> +b2, SiLU ->
                   segment-sum via matmul against an is_equal indicator,
                   accumulated in PSUM across the whole 128-node group ->
                   one contiguous store per group.
      node phase:  DMA-transpose loads of h and agg (feature-major), node MLP
                   + residual + chunk-local LayerNorm, and the next layer's
                   projected tables (with the x_t @ W1c fold done by a K=4
                   accumulate matmul).
      comm:        one AllGather of the src table only.
  - Host: final projection + MSE loss (tiny).
"""

import numpy as np
import ml_dtypes

BF16 = ml_dtypes.bfloat16
EPS = 1e-5
NCORES = 8
P = 128          # partition width / hidden size (H must equal 128)
GCH = 7          # groups gathered per indirect-DMA instruction pair
CW = 4           # node windows per node-phase chunk


def _silu(x):
    return x * (1.0 / (1.0 + np.exp(-x)))


# ----------------------------------------------------------------------------
# Host-side preprocessing
# ----------------------------------------------------------------------------

def _preprocess(inputs):
    pos0 = np.asarray(inputs["pos0"], np.float32)
    pos1 = np.asarray(inputs["pos1"], np.float32)
    z = np.asarray(inputs["z"], np.float32)
    t = np.asarray(inputs["t"], np.float32)
    edge_index = np.asarray(inputs["edge_index"])
    batch = np.asarray(inputs["batch"])
    ew1 = np.asarray(inputs["ew1"], np.float32)
    eb1 = np.asarray(inputs["eb1"], np.float32)
    ew2 = np.asarray(inputs["ew2"], np.float32)
    eb2 = np.asarray(inputs["eb2"], np.float32)
    nw1 = np.asarray(inputs["nw1"], np.float32)
    nb1 = np.asarray(inputs["nb1"], np.float32)
    nw2 = np.asarray(inputs["nw2"], np.float32)
    nb2 = np.asarray(inputs["nb2"], np.float32)
    ln_g = np.asarray(inputs["ln_g"], np.float32)
    ln_b = np.asarray(inputs["ln_b"], np.float32)

    V = pos0.shape[0]
    L = ew1.shape[0]
    H = ew1.shape[2]
    assert H == P
    nv = V // NCORES
    assert nv * NCORES == V
    ngrp = (nv + P - 1) // P
    nvp = ngrp * P

    ts = float(t[0])
    x_t = (1.0 - ts) * pos0 + ts * pos1
    target = pos1 - pos0

    te_w1 = np.asarray(inputs["te_w1"], np.float32)
    te_b1 = np.asarray(inputs["te_b1"], np.float32)
    te_w2 = np.asarray(inputs["te_w2"], np.float32)
    te_b2 = np.asarray(inputs["te_b2"], np.float32)
    cp_w = np.asarray(inputs["cp_w"], np.float32)
    cp_b = np.asarray(inputs["cp_b"], np.float32)

    t_emb = _silu(np.array([[ts]], np.float32) @ te_w1 + te_b1) @ te_w2 + te_b2
    h0 = np.concatenate(
        [z[batch], np.broadcast_to(t_emb, (V, t_emb.shape[1]))], axis=1
    ) @ cp_w + cp_b  # [V, H] f32

    # layer-0 folded tables (rel-pos + bias folded in; see module docstring)
    Y = x_t @ ew1[0, 2 * H:2 * H + 3]                 # [V, H]
    h1d0 = h0 @ ew1[0, :H] + Y + eb1[0]
    h1s0 = h0 @ ew1[0, H:2 * H] - Y

    def pad_rows(a):
        out = np.zeros((nvp, a.shape[1]), a.dtype)
        out[:a.shape[0]] = a
        return out

    h1s_full0 = np.concatenate(
        [pad_rows(h1s0[c * nv:(c + 1) * nv]) for c in range(NCORES)], axis=0
    ).astype(BF16)                                    # [8*nvp, H]

    # edges sorted by destination, split at core boundaries
    src_g = edge_index[0].astype(np.int64)
    dst_g = edge_index[1].astype(np.int64)
    order = np.argsort(dst_g, kind="stable")
    dst_s = dst_g[order]
    src_s = src_g[order]
    bounds = np.searchsorted(dst_s, np.arange(0, V + 1, nv))

    # uniform per-group edge capacity
    maxe = 0
    per_edges = []
    for c in range(NCORES):
        e0, e1 = int(bounds[c]), int(bounds[c + 1])
        ld = (dst_s[e0:e1] - c * nv).astype(np.int64)
        sg = src_s[e0:e1]
        grp = ld // P
        cnt = np.bincount(grp, minlength=ngrp)
        maxe = max(maxe, int(cnt.max()))
        per_edges.append((ld, sg, cnt))
    PPG = (maxe + P - 1) // P
    EG = PPG * P
    ncol = ngrp * PPG

    per_core = []
    for c in range(NCORES):
        ld, sg, cnt = per_edges[c]
        srcidx = np.zeros(ngrp * EG, np.int32)
        dstrel = np.full(ngrp * EG, -1, np.int32)
        starts = np.concatenate([[0], np.cumsum(cnt)])
        for g in range(ngrp):
            s0, s1 = int(starts[g]), int(starts[g + 1])
            n = s1 - s0
            sl = slice(g * EG, g * EG + n)
            srow = (sg[s0:s1] // nv) * nvp + (sg[s0:s1] % nv)
            srcidx[sl] = srow
            dstrel[sl] = ld[s0:s1] - g * P
        # slot (j, p) = edge j*128 + p  ->  column-major [P, ncol]
        per_core.append({
            "srcidx": srcidx.reshape(ncol, P).T.copy(),
            "dstrel": dstrel.reshape(ncol, P).T.copy(),
            "dstrelF": dstrel.astype(BF16)[None, :].copy(),
            "h1d_own0": pad_rows(h1d0[c * nv:(c + 1) * nv]).astype(BF16),
            "h0own": pad_rows(h0[c * nv:(c + 1) * nv]).astype(BF16),
            "x4own": np.concatenate(
                [pad_rows(x_t[c * nv:(c + 1) * nv]).T,
                 np.ones((1, nvp), np.float32)], axis=0),   # [4, nvp]
        })

    # device weight layouts (concat layers along free dim)
    cat = lambda m: np.concatenate([m[l] for l in range(L)], axis=1)
    bc = lambda v: np.concatenate(
        [np.broadcast_to(v[l], (P, H)) for l in range(L)], axis=1)
    w1a = cat(ew1[:, :H]).astype(BF16)
    w1b = cat(ew1[:, H:2 * H]).astype(BF16)
    w2 = cat(ew2).astype(BF16)
    w1c4d = np.concatenate(
        [np.concatenate([ew1[l, 2 * H:], eb1[l][None, :]], 0)
         for l in range(L)], axis=1).astype(np.float32)        # [4, L*H]
    w1c4s = np.concatenate(
        [np.concatenate([-ew1[l, 2 * H:], np.zeros((1, H), np.float32)], 0)
         for l in range(L)], axis=1).astype(np.float32)
    weights = dict(
        w1a=w1a, w1b=w1b, w2=w2, w1c4d=w1c4d, w1c4s=w1c4s,
        b2bc=bc(eb2).astype(np.float32),
        nw1h=cat(nw1[:, :H]).astype(BF16),
        nw1a=cat(nw1[:, H:]).astype(BF16),
        nw2=cat(nw2).astype(BF16),
        nb1c=nb1.T.astype(np.float32).copy(),                  # [H, L]
        nb2bc=bc(nb2).astype(np.float32),
        lngbc=bc(ln_g).astype(np.float32),
        lnbbc=bc(ln_b).astype(np.float32),
        ident=np.eye(P, dtype=BF16),
        iota=np.tile(np.arange(P, dtype=np.int32), (P, 1)),
        iotacolf=np.arange(P, dtype=np.float32)[:, None].copy(),
    )

    geom = dict(V=V, L=L, H=H, nv=nv, nvp=nvp, ngrp=ngrp, PPG=PPG)
    host = dict(h1s_full0=h1s_full0, target=target,
                op_w=np.asarray(inputs["op_w"], np.float32),
                op_b=np.asarray(inputs["op_b"], np.float32))
    return geom, per_core, weights, host


# ----------------------------------------------------------------------------
# Device program
# ----------------------------------------------------------------------------

SILU_DECOMPOSED = False  # sim has no Silu table; set True for CoreSim runs


def _build_program(geom):
    import concourse.bass as bass
    import concourse.bacc as bacc
    import concourse.mybir as mybir
    import concourse.tile as tile

    dt = mybir.dt
    AF = mybir.ActivationFunctionType
    ALU = mybir.AluOpType
    IOA = bass.IndirectOffsetOnAxis
    AX = mybir.AxisListType

    L, nvp, ngrp, PPG = geom["L"], geom["nvp"], geom["ngrp"], geom["PPG"]
    ncol = ngrp * PPG
    NW = ngrp  # node windows per core

    nc = bacc.Bacc(num_devices=NCORES)

    # ---- I/O ----
    h1sf0 = nc.declare_dram_parameter("h1s_full0", [NCORES * nvp, P],
                                      dt.bfloat16, isOutput=False)
    h1d0_d = nc.declare_dram_parameter("h1d_own0", [nvp, P], dt.bfloat16,
                                       isOutput=False)
    h0own = nc.declare_dram_parameter("h0own", [nvp, P], dt.bfloat16,
                                      isOutput=False)
    srcidx_d = nc.declare_dram_parameter("srcidx", [P, ncol], dt.int32, isOutput=False)
    dstrel_d = nc.declare_dram_parameter("dstrel", [P, ncol], dt.int32, isOutput=False)
    dstrelF_d = nc.declare_dram_parameter("dstrelF", [1, ncol * P], dt.bfloat16, isOutput=False)
    x4own_d = nc.declare_dram_parameter("x4own", [4, nvp], dt.float32, isOutput=False)
    w1a_d = nc.declare_dram_parameter("w1a", [P, L * P], dt.bfloat16, isOutput=False)
    w1b_d = nc.declare_dram_parameter("w1b", [P, L * P], dt.bfloat16, isOutput=False)
    w2_d = nc.declare_dram_parameter("w2", [P, L * P], dt.bfloat16, isOutput=False)
    w1c4d_d = nc.declare_dram_parameter("w1c4d", [4, L * P], dt.float32, isOutput=False)
    w1c4s_d = nc.declare_dram_parameter("w1c4s", [4, L * P], dt.float32, isOutput=False)
    b2bc_d = nc.declare_dram_parameter("b2bc", [P, L * P], dt.float32, isOutput=False)
    nw1h_d = nc.declare_dram_parameter("nw1h", [P, L * P], dt.bfloat16, isOutput=False)
    nw1a_d = nc.declare_dram_parameter("nw1a", [P, L * P], dt.bfloat16, isOutput=False)
    nw2_d = nc.declare_dram_parameter("nw2", [P, L * P], dt.bfloat16, isOutput=False)
    nb1c_d = nc.declare_dram_parameter("nb1c", [P, L], dt.float32, isOutput=False)
    nb2bc_d = nc.declare_dram_parameter("nb2bc", [P, L * P], dt.float32, isOutput=False)
    lngbc_d = nc.declare_dram_parameter("lngbc", [P, L * P], dt.float32, isOutput=False)
    lnbbc_d = nc.declare_dram_parameter("lnbbc", [P, L * P], dt.float32, isOutput=False)
    ident_d = nc.declare_dram_parameter("ident", [P, P], dt.bfloat16, isOutput=False)
    iota_d = nc.declare_dram_parameter("iota", [P, P], dt.int32, isOutput=False)
    iotacolf_d = nc.declare_dram_parameter("iotacolf", [P, 1], dt.float32, isOutput=False)
    hout_d = nc.declare_dram_parameter("hout", [nvp, P], dt.float32, isOutput=True)

    # ---- internal DRAM ----
    agg_hbm = nc.dram_tensor("agg_hbm", [nvp, P], dt.bfloat16)
    hown = [nc.dram_tensor(f"hown{l + 1}", [nvp, P], dt.bfloat16)
            for l in range(L - 1)]
    h1down = [nc.dram_tensor(f"h1down{l + 1}", [nvp, P], dt.bfloat16)
              for l in range(L - 1)]
    h1sown = [nc.dram_tensor(f"h1sown{l + 1}", [nvp, P], dt.bfloat16)
              for l in range(L - 1)]
    h1sfull = [nc.dram_tensor(f"h1sfull{l + 1}", [NCORES * nvp, P],
                              dt.bfloat16, addr_space="Shared")
               for l in range(L - 1)]

    groups = [list(range(NCORES))]

    with tile.TileContext(nc) as tc:
        with (
            tc.tile_pool(name="const", bufs=1) as cpool,
            tc.tile_pool(name="gather", bufs=2) as gpool,
            tc.tile_pool(name="gsrc", bufs=6) as gspool,
            tc.tile_pool(name="work", bufs=3) as wpool,
            tc.tile_pool(name="small", bufs=4) as spool,
            tc.tile_pool(name="pmB", bufs=6, space="PSUM") as pmB,
            tc.tile_pool(name="pagg", bufs=2, space="PSUM") as pagg,
        ):
            def cload(src, shape, dtype, tag):
                t_ = cpool.tile(shape, dtype, tag=tag)
                nc.sync.dma_start(out=t_[:], in_=src[:, :])
                return t_

            identsb = cload(ident_d, [P, P], dt.bfloat16, "ident")
            iotasb = cload(iota_d, [P, P], dt.int32, "iota")
            onesb = cpool.tile([1, P], dt.bfloat16, tag="ones")
            nc.vector.memset(onesb[:], 1.0)
            iotacol = cload(iotacolf_d, [P, 1], dt.float32, "iotacol")
            srcidxsb = cload(srcidx_d, [P, ncol], dt.int32, "srcidx")
            dstrelsb = cload(dstrel_d, [P, ncol], dt.int32, "dstrel")
            x4sb = cload(x4own_d, [4, nvp], dt.float32, "x4own")
            w1asb = cload(w1a_d, [P, L * P], dt.bfloat16, "w1a")
            w1bsb = cload(w1b_d, [P, L * P], dt.bfloat16, "w1b")
            w2sb = cload(w2_d, [P, L * P], dt.bfloat16, "w2")
            w1c4dsb = cload(w1c4d_d, [4, L * P], dt.float32, "w1c4d")
            w1c4ssb = cload(w1c4s_d, [4, L * P], dt.float32, "w1c4s")
            b2bcsb = cload(b2bc_d, [P, L * P], dt.float32, "b2bc")
            nw1hsb = cload(nw1h_d, [P, L * P], dt.bfloat16, "nw1h")
            nw1asb = cload(nw1a_d, [P, L * P], dt.bfloat16, "nw1a")
            nw2sb = cload(nw2_d, [P, L * P], dt.bfloat16, "nw2")
            nb1csb = cload(nb1c_d, [P, L], dt.float32, "nb1c")
            nb2bcsb = cload(nb2bc_d, [P, L * P], dt.float32, "nb2bc")
            lngbcsb = cload(lngbc_d, [P, L * P], dt.float32, "lngbc")
            lnbbcsb = cload(lnbbc_d, [P, L * P], dt.float32, "lnbbc")

            def emit_silu(out_ap, in_ap, scratch_pool, tag, bias=0.0):
                if not SILU_DECOMPOSED:
                    nc.scalar.activation(out_ap, in_ap, AF.Silu, bias=bias)
                else:
                    sg = scratch_pool.tile(
                        [P, in_ap.shape[-1] if in_ap.ndim == 2 else P],
                        dt.float32, tag=tag)
                    sga = sg[:in_ap.shape[0], :in_ap.shape[-1]]
                    nc.scalar.activation(sga, in_ap, AF.Sigmoid, bias=bias)
                    nc.vector.tensor_tensor(out=out_ap, in0=in_ap, in1=sga,
                                            op=ALU.mult)

            def edge_phase(l, h1d_dram, h1s_dram):
                lsl = slice(l * P, (l + 1) * P)
                for g0 in range(0, ngrp, GCH):
                    g1 = min(g0 + GCH, ngrp)
                    nrows = (g1 - g0) * P
                    # dst node rows for these groups (contiguous, node-major)
                    nodes = gpool.tile([P, GCH * P], dt.bfloat16, tag="nodes")
                    nc.sync.dma_start(
                        out=nodes[:, :nrows].rearrange("p (g h) -> p g h", h=P),
                        in_=h1d_dram[g0 * P:g1 * P, :].rearrange(
                            "(g p) h -> p g h", p=P))
                    # per-edge slot ids along the free axis (for S_T build)
                    relf = gpool.tile([1, GCH * PPG * P], dt.bfloat16, tag="relf")
                    nc.sync.dma_start(
                        out=relf[:, :(g1 - g0) * PPG * P],
                        in_=dstrelF_d[:, g0 * PPG * P:g1 * PPG * P])
                    for g in range(g0, g1):
                        aggp = pagg.tile([P, P], dt.float32, tag="agg")
                        nodes_g = nodes[:, (g - g0) * P:(g - g0 + 1) * P]
                        for js in range(0, PPG, 4):
                            je = min(js + 4, PPG)
                            nsb = je - js
                            # src rows: one indirect gather per 128-edge panel
                            gs = gspool.tile([P, 4 * P], dt.bfloat16, tag="gs")
                            for k in range(nsb):
                                col = g * PPG + js + k
                                nc.gpsimd.indirect_dma_start(
                                    out=gs[:, k * P:(k + 1) * P],
                                    out_offset=None,
                                    in_=h1s_dram[:, :],
                                    in_offset=IOA(ap=srcidxsb[:, col:col + 1],
                                                  axis=0))
                            # S_T[slot, e] via ones-matmul broadcast + is_equal
                            rsl = slice(((g - g0) * PPG + js) * P,
                                        ((g - g0) * PPG + je) * P)
                            bcp = pmB.tile([P, 4 * P], dt.float32, tag="B")
                            nc.tensor.matmul(bcp[:, :nsb * P], lhsT=onesb[:],
                                             rhs=relf[:, rsl],
                                             start=True, stop=True)
                            ST = wpool.tile([P, 4 * P], dt.bfloat16, tag="ST")
                            nc.vector.tensor_scalar(
                                ST[:, :nsb * P], bcp[:, :nsb * P],
                                iotacol[:], None, op0=ALU.is_equal)
                            # m1t (feature-major, PSUM): gathered src rows are
                            # transposed in via matmul-with-identity, the dst
                            # expansion streams S_T against the node tile
                            m1tp = pmB.tile([P, 4 * P], dt.float32, tag="B")
                            for k in range(nsb):
                                ksl = slice(k * P, (k + 1) * P)
                                nc.tensor.matmul(
                                    m1tp[:, ksl], lhsT=gs[:, ksl],
                                    rhs=identsb[:], start=True, stop=False,
                                    skip_group_check=True)
                                nc.tensor.matmul(
                                    m1tp[:, ksl], lhsT=nodes_g,
                                    rhs=ST[:, ksl], start=False, stop=True,
                                    skip_group_check=True)
                            m1t = wpool.tile([P, 4 * P], dt.bfloat16, tag="m1tsb")
                            emit_silu(m1t[:, :nsb * P], m1tp[:, :nsb * P],
                                      wpool, "sg1")
                            m2p = pmB.tile([P, 4 * P], dt.float32, tag="B")
                            for k in range(nsb):
                                nc.tensor.matmul(
                                    m2p[:, k * P:(k + 1) * P],
                                    lhsT=m1t[:, k * P:(k + 1) * P],
                                    rhs=w2sb[:, lsl], start=True, stop=True)
                            nc.vector.tensor_tensor(
                                out=m2p[:, :nsb * P].rearrange(
                                    "p (j h) -> p j h", h=P),
                                in0=m2p[:, :nsb * P].rearrange(
                                    "p (j h) -> p j h", h=P),
                                in1=b2bcsb[:, lsl].unsqueeze(1)
                                    .to_broadcast([P, nsb, P]),
                                op=ALU.add)
                            m2s = wpool.tile([P, 4 * P], dt.bfloat16, tag="m2s")
                            emit_silu(m2s[:, :nsb * P], m2p[:, :nsb * P],
                                      wpool, "sg2")
                            S = wpool.tile([P, 4 * P], dt.bfloat16, tag="S")
                            csl = slice(g * PPG + js, g * PPG + je)
                            nc.vector.tensor_tensor(
                                out=S[:, :nsb * P].rearrange(
                                    "p (j s) -> p j s", s=P),
                                in0=dstrelsb[:, csl].unsqueeze(2)
                                    .to_broadcast([P, nsb, P]),
                                in1=iotasb[:].unsqueeze(1)
                                    .to_broadcast([P, nsb, P]),
                                op=ALU.is_equal)
                            for k in range(nsb):
                                nc.tensor.matmul(
                                    aggp[:], lhsT=S[:, k * P:(k + 1) * P],
                                    rhs=m2s[:, k * P:(k + 1) * P],
                                    start=(js == 0 and k == 0),
                                    stop=(je == PPG and k == nsb - 1),
                                    skip_group_check=True)
                        aggsb = spool.tile([P, P], dt.bfloat16, tag="aggsb")
                        nc.scalar.activation(aggsb[:], aggp[:], AF.Copy)
                        nc.sync.dma_start(out=agg_hbm[g * P:(g + 1) * P, :],
                                          in_=aggsb[:])

            def node_phase(l, hprev, hnext, h1d_next, h1s_next):
                lsl = slice(l * P, (l + 1) * P)
                last = l == L - 1
                for w0 in range(0, NW, CW):
                    w1 = min(w0 + CW, NW)
                    nw = w1 - w0
                    cnt = nw * P
                    rows = slice(w0 * P, w0 * P + cnt)
                    hwt = spool.tile([P, CW * P], dt.bfloat16, tag="hwt")
                    nc.sync.dma_start_transpose(hwt[:, :cnt], hprev[rows, :])
                    awt = spool.tile([P, CW * P], dt.bfloat16, tag="awt")
                    nc.sync.dma_start_transpose(awt[:, :cnt], agg_hbm[rows, :])
                    hwin = spool.tile([P, CW * P], dt.bfloat16, tag="hwin")
                    nc.sync.dma_start(
                        out=hwin[:, :cnt].rearrange("p (w h) -> p w h", h=P),
                        in_=hprev[rows, :].rearrange("(w p) h -> p w h", p=P))

                    n1p = pmB.tile([P, CW * P], dt.float32, tag="B")
                    nc.tensor.matmul(n1p[:, :cnt], lhsT=nw1hsb[:, lsl],
                                     rhs=hwt[:, :cnt], start=True, stop=False)
                    nc.tensor.matmul(n1p[:, :cnt], lhsT=nw1asb[:, lsl],
                                     rhs=awt[:, :cnt], start=False, stop=True)
                    n1s = spool.tile([P, CW * P], dt.bfloat16, tag="n1s")
                    emit_silu(n1s[:, :cnt], n1p[:, :cnt], spool, "sgn",
                              bias=nb1csb[:, l:l + 1])

                    n2p = pmB.tile([P, CW * P], dt.float32, tag="B")
                    for w in range(nw):
                        nc.tensor.matmul(n2p[:, w * P:(w + 1) * P],
                                         lhsT=n1s[:, w * P:(w + 1) * P],
                                         rhs=nw2sb[:, lsl],
                                         start=True, stop=True)
                    nc.vector.tensor_tensor(
                        out=n2p[:, :cnt].rearrange("p (w h) -> p w h", h=P),
                        in0=n2p[:, :cnt].rearrange("p (w h) -> p w h", h=P),
                        in1=nb2bcsb[:, lsl].unsqueeze(1)
                            .to_broadcast([P, nw, P]),
                        op=ALU.add)
                    nc.vector.tensor_tensor(
                        out=n2p[:, :cnt].rearrange("p (w h) -> p w h", h=P),
                        in0=n2p[:, :cnt].rearrange("p (w h) -> p w h", h=P),
                        in1=hwin[:, :cnt].rearrange("p (w h) -> p w h", h=P),
                        op=ALU.add)

                    mu = spool.tile([P, CW], dt.float32, tag="mu")
                    var = spool.tile([P, CW], dt.float32, tag="var")
                    sq = spool.tile([P, P], dt.float32, tag="sq")
                    for w in range(nw):
                        wsl = slice(w * P, (w + 1) * P)
                        nc.vector.reduce_sum(mu[:, w:w + 1], n2p[:, wsl],
                                             axis=AX.X)
                    nc.vector.tensor_scalar_mul(mu[:, :nw], mu[:, :nw], 1.0 / P)
                    for w in range(nw):
                        wsl = slice(w * P, (w + 1) * P)
                        nc.vector.tensor_scalar_sub(n2p[:, wsl], n2p[:, wsl],
                                                    mu[:, w:w + 1])
                        nc.scalar.activation(sq[:], n2p[:, wsl], AF.Square,
                                             accum_out=var[:, w:w + 1])
                    nc.vector.tensor_scalar(var[:, :nw], var[:, :nw],
                                            1.0 / P, EPS,
                                            op0=ALU.mult, op1=ALU.add)
                    srt = spool.tile([P, CW], dt.float32, tag="srt")
                    nc.scalar.activation(srt[:, :nw], var[:, :nw], AF.Sqrt)
                    rstd = spool.tile([P, CW], dt.float32, tag="rstd")
                    nc.vector.reciprocal(rstd[:, :nw], srt[:, :nw])

                    xn = spool.tile([P, CW * P], dt.float32, tag="xn")
                    for w in range(nw):
                        wsl = slice(w * P, (w + 1) * P)
                        nc.vector.tensor_scalar_mul(xn[:, wsl], n2p[:, wsl],
                                                    rstd[:, w:w + 1])
                    nc.vector.tensor_tensor(
                        out=xn[:, :cnt].rearrange("p (w h) -> p w h", h=P),
                        in0=xn[:, :cnt].rearrange("p (w h) -> p w h", h=P),
                        in1=lngbcsb[:, lsl].unsqueeze(1)
                            .to_broadcast([P, nw, P]),
                        op=ALU.mult)
                    nc.vector.tensor_tensor(
                        out=xn[:, :cnt].rearrange("p (w h) -> p w h", h=P),
                        in0=xn[:, :cnt].rearrange("p (w h) -> p w h", h=P),
                        in1=lnbbcsb[:, lsl].unsqueeze(1)
                            .to_broadcast([P, nw, P]),
                        op=ALU.add)

                    if last:
                        nc.sync.dma_start(
                            out=hout_d[rows, :].rearrange(
                                "(w p) h -> p w h", p=P),
                            in_=xn[:, :cnt].rearrange("p (w h) -> p w h", h=P))
                        continue

                    hnb = spool.tile([P, CW * P], dt.bfloat16, tag="hnb")
                    nc.vector.tensor_copy(hnb[:, :cnt], xn[:, :cnt])
                    nc.sync.dma_start(
                        out=hnext[rows, :].rearrange("(w p) h -> p w h", p=P),
                        in_=hnb[:, :cnt].rearrange("p (w h) -> p w h", h=P))
                    hnt = spool.tile([P, CW * P], dt.bfloat16, tag="hnt")
                    nc.sync.dma_start_transpose(hnt[:, :cnt], hnext[rows, :])

                    nsl = slice((l + 1) * P, (l + 2) * P)
                    for (tbl, wab, w1c4b, tag) in (
                        (h1d_next, w1asb, w1c4dsb, "pd"),
                        (h1s_next, w1bsb, w1c4ssb, "ps"),
                    ):
                        pp = pmB.tile([P, CW * P], dt.float32, tag="B")
                        for w in range(nw):
                            wsl = slice(w * P, (w + 1) * P)
                            nc.tensor.matmul(pp[:, wsl],
                                             lhsT=hnt[:, wsl],
                                             rhs=wab[:, nsl],
                                             start=True, stop=False)
                            nc.tensor.matmul(
                                pp[:, wsl],
                                lhsT=x4sb[:, w0 * P + w * P:
                                          w0 * P + (w + 1) * P],
                                rhs=w1c4b[:, nsl],
                                start=False, stop=True)
                        pb = spool.tile([P, CW * P], dt.bfloat16, tag=tag)
                        nc.scalar.activation(pb[:, :cnt], pp[:, :cnt], AF.Copy)
                        nc.sync.dma_start(
                            out=tbl[rows, :].rearrange("(w p) h -> p w h", p=P),
                            in_=pb[:, :cnt].rearrange("p (w h) -> p w h", h=P))

            for l in range(L):
                h1d_dram = h1d0_d if l == 0 else h1down[l - 1]
                h1s_dram = h1sf0 if l == 0 else h1sfull[l - 1]
                hprev = h0own if l == 0 else hown[l - 1]
                edge_phase(l, h1d_dram, h1s_dram)
                if l < L - 1:
                    node_phase(l, hprev, hown[l], h1down[l], h1sown[l])
                    nc.gpsimd.collective_compute(
                        "AllGather", mybir.AluOpType.bypass,
                        replica_groups=groups,
                        ins=[h1sown[l][:, :]], outs=[h1sfull[l][:, :]])
                else:
                    node_phase(l, hprev, None, None, None)

    nc.finalize()
    return nc


# ----------------------------------------------------------------------------
# Entry point
# ----------------------------------------------------------------------------

def _make_in_maps(geom, per_core, weights, host):
    in_maps = []
    for c in range(NCORES):
        pc = per_core[c]
        m = {
            "h1s_full0": host["h1s_full0"],
            "h1d_own0": pc["h1d_own0"],
            "h0own": pc["h0own"],
            "srcidx": pc["srcidx"], "dstrel": pc["dstrel"],
            "dstrelF": pc["dstrelF"], "x4own": pc["x4own"],
        }
        m.update(weights)
        in_maps.append(m)
    return in_maps


def _postprocess(geom, host, houts):
    nv = geom["nv"]
    h = np.concatenate([ho[:nv] for ho in houts], axis=0).astype(np.float32)
    v_pred = h @ host["op_w"] + host["op_b"]
    diff = v_pred - host["target"]
    return np.float32(np.mean(diff.astype(np.float64) ** 2))


def kernel(**inputs):
    from concourse.bass_utils import run_bass_kernel_spmd

    geom, per_core, weights, host = _preprocess(inputs)
    nc = _build_program(geom)
    in_maps = _make_in_maps(geom, per_core, weights, host)
    res = run_bass_kernel_spmd(nc, in_maps, list(range(NCORES)))
    houts = [res.results[c]["hout"] for c in range(NCORES)]
    return _postprocess(geom, host, houts)


# revision 11
# speedup vs baseline: 1.8822x; 1.0672x over previous
"""Trainium2 Bass kernel for an MLP flow-matching GNN (message passing).

Strategy (8 NeuronCores, SPMD):
  - Host: sort edges by destination, partition nodes into 8 contiguous ranges
    (one per core), pad each core's node count to a multiple of 128.  Nodes are
    processed in UNIFORM groups of 128; each group's incident edges are padded
    to a uniform per-group edge capacity (EG = PPG*128, PPG = max group degree
    sum over all cores, in panels of 128).  This makes every slice bound
    core-uniform so a single SPMD program works for all cores.
  - The edge-MLP first layer is pre-projected into per-node tables:
        h1d = h @ W1a + x_t @ W1c + b1      (dst table, local per core)
        h1s = h @ W1b - x_t @ W1c           (src table, AllGathered)
    so that m1[e] = h1d[dst] + h1s[src] includes the relative-position term
    (rel @ W1c = Y[dst] - Y[src]) and bias with no per-edge matmul.
  - Device, per layer:
      edge phase:  two batched indirect-DMA gathers per chunk of groups (dst
                   rows, then src rows accumulated via the DMA CCE add) ->
                   SiLU -> PE transpose -> edge-MLP2 matmul -> +b2, SiLU ->
                   segment-sum via matmul against an is_equal indicator,
                   accumulated in PSUM across the whole 128-node group ->
                   one contiguous store per group.
      node phase:  DMA-transpose loads of h and agg (feature-major), node MLP
                   + residual + chunk-local LayerNorm, and the next layer's
                   projected tables (with the x_t @ W1c fold done by a K=4
                   accumulate matmul).
      comm:        one AllGather of the src table only.
  - Host: final projection + MSE loss (tiny).
"""

import numpy as np
import ml_dtypes

BF16 = ml_dtypes.bfloat16
EPS = 1e-5
NCORES = 8
P = 128          # partition width / hidden size (H must equal 128)
GCH = 7          # groups gathered per indirect-DMA instruction pair
CW = 4           # node windows per node-phase chunk


def _silu(x):
    return x * (1.0 / (1.0 + np.exp(-x)))


# ----------------------------------------------------------------------------
# Host-side preprocessing
# ----------------------------------------------------------------------------

def _preprocess(inputs):
    pos0 = np.asarray(inputs["pos0"], np.float32)
    pos1 = np.asarray(inputs["pos1"], np.float32)
    z = np.asarray(inputs["z"], np.float32)
    t = np.asarray(inputs["t"], np.float32)
    edge_index = np.asarray(inputs["edge_index"])
    batch = np.asarray(inputs["batch"])
    ew1 = np.asarray(inputs["ew1"], np.float32)
    eb1 = np.asarray(inputs["eb1"], np.float32)
    ew2 = np.asarray(inputs["ew2"], np.float32)
    eb2 = np.asarray(inputs["eb2"], np.float32)
    nw1 = np.asarray(inputs["nw1"], np.float32)
    nb1 = np.asarray(inputs["nb1"], np.float32)
    nw2 = np.asarray(inputs["nw2"], np.float32)
    nb2 = np.asarray(inputs["nb2"], np.float32)
    ln_g = np.asarray(inputs["ln_g"], np.float32)
    ln_b = np.asarray(inputs["ln_b"], np.float32)

    V = pos0.shape[0]
    L = ew1.shape[0]
    H = ew1.shape[2]
    assert H == P
    nv = V // NCORES
    assert nv * NCORES == V
    ngrp = (nv + P - 1) // P
    nvp = ngrp * P

    ts = float(t[0])
    x_t = (1.0 - ts) * pos0 + ts * pos1
    target = pos1 - pos0

    te_w1 = np.asarray(inputs["te_w1"], np.float32)
    te_b1 = np.asarray(inputs["te_b1"], np.float32)
    te_w2 = np.asarray(inputs["te_w2"], np.float32)
    te_b2 = np.asarray(inputs["te_b2"], np.float32)
    cp_w = np.asarray(inputs["cp_w"], np.float32)
    cp_b = np.asarray(inputs["cp_b"], np.float32)

    t_emb = _silu(np.array([[ts]], np.float32) @ te_w1 + te_b1) @ te_w2 + te_b2
    h0 = np.concatenate(
        [z[batch], np.broadcast_to(t_emb, (V, t_emb.shape[1]))], axis=1
    ) @ cp_w + cp_b  # [V, H] f32

    # layer-0 folded tables (rel-pos + bias folded in; see module docstring)
    Y = x_t @ ew1[0, 2 * H:2 * H + 3]                 # [V, H]
    h1d0 = h0 @ ew1[0, :H] + Y + eb1[0]
    h1s0 = h0 @ ew1[0, H:2 * H] - Y

    def pad_rows(a):
        out = np.zeros((nvp, a.shape[1]), a.dtype)
        out[:a.shape[0]] = a
        return out

    h1s_full0 = np.concatenate(
        [pad_rows(h1s0[c * nv:(c + 1) * nv]) for c in range(NCORES)], axis=0
    ).astype(BF16)                                    # [8*nvp, H]

    # edges sorted by destination, split at core boundaries
    src_g = edge_index[0].astype(np.int64)
    dst_g = edge_index[1].astype(np.int64)
    order = np.argsort(dst_g, kind="stable")
    dst_s = dst_g[order]
    src_s = src_g[order]
    bounds = np.searchsorted(dst_s, np.arange(0, V + 1, nv))

    # uniform per-group edge capacity
    maxe = 0
    per_edges = []
    for c in range(NCORES):
        e0, e1 = int(bounds[c]), int(bounds[c + 1])
        ld = (dst_s[e0:e1] - c * nv).astype(np.int64)
        sg = src_s[e0:e1]
        grp = ld // P
        cnt = np.bincount(grp, minlength=ngrp)
        maxe = max(maxe, int(cnt.max()))
        per_edges.append((ld, sg, cnt))
    PPG = (maxe + P - 1) // P
    EG = PPG * P
    ncol = ngrp * PPG

    per_core = []
    for c in range(NCORES):
        ld, sg, cnt = per_edges[c]
        srcidx = np.zeros(ngrp * EG, np.int32)
        dstrel = np.full(ngrp * EG, -1, np.int32)
        starts = np.concatenate([[0], np.cumsum(cnt)])
        for g in range(ngrp):
            s0, s1 = int(starts[g]), int(starts[g + 1])
            n = s1 - s0
            sl = slice(g * EG, g * EG + n)
            srow = (sg[s0:s1] // nv) * nvp + (sg[s0:s1] % nv)
            srcidx[sl] = srow
            dstrel[sl] = ld[s0:s1] - g * P
        # slot (j, p) = edge j*128 + p  ->  column-major [P, ncol]
        per_core.append({
            "srcidx": srcidx.reshape(ncol, P).T.copy(),
            "dstrel": dstrel.reshape(ncol, P).T.copy(),
            "dstrelF": dstrel.astype(BF16)[None, :].copy(),
            "h1d_own0": pad_rows(h1d0[c * nv:(c + 1) * nv]).astype(BF16),
            "h0own": pad_rows(h0[c * nv:(c + 1) * nv]).astype(BF16),
            "x4own": np.concatenate(
                [pad_rows(x_t[c * nv:(c + 1) * nv]).T,
                 np.ones((1, nvp), np.float32)], axis=0),   # [4, nvp]
        })

    # device weight layouts (concat layers along free dim)
    cat = lambda m: np.concatenate([m[l] for l in range(L)], axis=1)
    bc = lambda v: np.concatenate(
        [np.broadcast_to(v[l], (P, H)) for l in range(L)], axis=1)
    w1a = cat(ew1[:, :H]).astype(BF16)
    w1b = cat(ew1[:, H:2 * H]).astype(BF16)
    w2 = cat(ew2).astype(BF16)
    w1c4d = np.concatenate(
        [np.concatenate([ew1[l, 2 * H:], eb1[l][None, :]], 0)
         for l in range(L)], axis=1).astype(np.float32)        # [4, L*H]
    w1c4s = np.concatenate(
        [np.concatenate([-ew1[l, 2 * H:], np.zeros((1, H), np.float32)], 0)
         for l in range(L)], axis=1).astype(np.float32)
    weights = dict(
        w1a=w1a, w1b=w1b, w2=w2, w1c4d=w1c4d, w1c4s=w1c4s,
        b2bc=bc(eb2).astype(np.float32),
        nw1h=cat(nw1[:, :H]).astype(BF16),
        nw1a=cat(nw1[:, H:]).astype(BF16),
        nw2=cat(nw2).astype(BF16),
        nb1c=nb1.T.astype(np.float32).copy(),                  # [H, L]
        nb2bc=bc(nb2).astype(np.float32),
        lngbc=bc(ln_g).astype(np.float32),
        lnbbc=bc(ln_b).astype(np.float32),
        ident=np.eye(P, dtype=BF16),
        iota=np.tile(np.arange(P, dtype=np.int32), (P, 1)),
        iotacolf=np.arange(P, dtype=np.float32)[:, None].copy(),
    )

    geom = dict(V=V, L=L, H=H, nv=nv, nvp=nvp, ngrp=ngrp, PPG=PPG)
    host = dict(h1s_full0=h1s_full0, target=target,
                op_w=np.asarray(inputs["op_w"], np.float32),
                op_b=np.asarray(inputs["op_b"], np.float32))
    return geom, per_core, weights, host


# ----------------------------------------------------------------------------
# Device program
# ----------------------------------------------------------------------------

SILU_DECOMPOSED = False  # sim has no Silu table; set True for CoreSim runs


def _build_program(geom):
    import concourse.bass as bass
    import concourse.bacc as bacc
    import concourse.mybir as mybir
    import concourse.tile as tile

    dt = mybir.dt
    AF = mybir.ActivationFunctionType
    ALU = mybir.AluOpType
    IOA = bass.IndirectOffsetOnAxis
    AX = mybir.AxisListType

    L, nvp, ngrp, PPG = geom["L"], geom["nvp"], geom["ngrp"], geom["PPG"]
    ncol = ngrp * PPG
    NW = ngrp  # node windows per core

    nc = bacc.Bacc(num_devices=NCORES)

    # ---- I/O ----
    h1sf0 = nc.declare_dram_parameter("h1s_full0", [NCORES * nvp, P],
                                      dt.bfloat16, isOutput=False)
    h1d0_d = nc.declare_dram_parameter("h1d_own0", [nvp, P], dt.bfloat16,
                                       isOutput=False)
    h0own = nc.declare_dram_parameter("h0own", [nvp, P], dt.bfloat16,
                                      isOutput=False)
    srcidx_d = nc.declare_dram_parameter("srcidx", [P, ncol], dt.int32, isOutput=False)
    dstrel_d = nc.declare_dram_parameter("dstrel", [P, ncol], dt.int32, isOutput=False)
    dstrelF_d = nc.declare_dram_parameter("dstrelF", [1, ncol * P], dt.bfloat16, isOutput=False)
    x4own_d = nc.declare_dram_parameter("x4own", [4, nvp], dt.float32, isOutput=False)
    w1a_d = nc.declare_dram_parameter("w1a", [P, L * P], dt.bfloat16, isOutput=False)
    w1b_d = nc.declare_dram_parameter("w1b", [P, L * P], dt.bfloat16, isOutput=False)
    w2_d = nc.declare_dram_parameter("w2", [P, L * P], dt.bfloat16, isOutput=False)
    w1c4d_d = nc.declare_dram_parameter("w1c4d", [4, L * P], dt.float32, isOutput=False)
    w1c4s_d = nc.declare_dram_parameter("w1c4s", [4, L * P], dt.float32, isOutput=False)
    b2bc_d = nc.declare_dram_parameter("b2bc", [P, L * P], dt.float32, isOutput=False)
    nw1h_d = nc.declare_dram_parameter("nw1h", [P, L * P], dt.bfloat16, isOutput=False)
    nw1a_d = nc.declare_dram_parameter("nw1a", [P, L * P], dt.bfloat16, isOutput=False)
    nw2_d = nc.declare_dram_parameter("nw2", [P, L * P], dt.bfloat16, isOutput=False)
    nb1c_d = nc.declare_dram_parameter("nb1c", [P, L], dt.float32, isOutput=False)
    nb2bc_d = nc.declare_dram_parameter("nb2bc", [P, L * P], dt.float32, isOutput=False)
    lngbc_d = nc.declare_dram_parameter("lngbc", [P, L * P], dt.float32, isOutput=False)
    lnbbc_d = nc.declare_dram_parameter("lnbbc", [P, L * P], dt.float32, isOutput=False)
    ident_d = nc.declare_dram_parameter("ident", [P, P], dt.bfloat16, isOutput=False)
    iota_d = nc.declare_dram_parameter("iota", [P, P], dt.int32, isOutput=False)
    iotacolf_d = nc.declare_dram_parameter("iotacolf", [P, 1], dt.float32, isOutput=False)
    hout_d = nc.declare_dram_parameter("hout", [nvp, P], dt.float32, isOutput=True)

    # ---- internal DRAM ----
    agg_hbm = [nc.dram_tensor(f"agg_hbm{i}", [nvp, P], dt.bfloat16)
               for i in range(2)]
    hown = [nc.dram_tensor(f"hown{l + 1}", [nvp, P], dt.bfloat16)
            for l in range(L - 1)]
    h1down = [nc.dram_tensor(f"h1down{l + 1}", [nvp, P], dt.bfloat16)
              for l in range(L - 1)]
    h1sown = [nc.dram_tensor(f"h1sown{l + 1}", [nvp, P], dt.bfloat16)
              for l in range(L - 1)]
    h1sfull = [nc.dram_tensor(f"h1sfull{l + 1}", [NCORES * nvp, P],
                              dt.bfloat16, addr_space="Shared")
               for l in range(L - 1)]

    groups = [list(range(NCORES))]

    with tile.TileContext(nc) as tc:
        with (
            tc.tile_pool(name="const", bufs=1) as cpool,
            tc.tile_pool(name="gather", bufs=2) as gpool,
            tc.tile_pool(name="gsrc", bufs=6) as gspool,
            tc.tile_pool(name="work", bufs=3) as wpool,
            tc.tile_pool(name="small", bufs=4) as spool,
            tc.tile_pool(name="pmB", bufs=6, space="PSUM") as pmB,
            tc.tile_pool(name="pagg", bufs=2, space="PSUM") as pagg,
        ):
            def cload(src, shape, dtype, tag):
                t_ = cpool.tile(shape, dtype, tag=tag)
                nc.sync.dma_start(out=t_[:], in_=src[:, :])
                return t_

            identsb = cload(ident_d, [P, P], dt.bfloat16, "ident")
            iotasb = cload(iota_d, [P, P], dt.int32, "iota")
            onesb = cpool.tile([1, P], dt.bfloat16, tag="ones")
            nc.vector.memset(onesb[:], 1.0)
            iotacol = cload(iotacolf_d, [P, 1], dt.float32, "iotacol")
            srcidxsb = cload(srcidx_d, [P, ncol], dt.int32, "srcidx")
            dstrelsb = cload(dstrel_d, [P, ncol], dt.int32, "dstrel")
            x4sb = cload(x4own_d, [4, nvp], dt.float32, "x4own")
            w1asb = cload(w1a_d, [P, L * P], dt.bfloat16, "w1a")
            w1bsb = cload(w1b_d, [P, L * P], dt.bfloat16, "w1b")
            w2sb = cload(w2_d, [P, L * P], dt.bfloat16, "w2")
            w1c4dsb = cload(w1c4d_d, [4, L * P], dt.float32, "w1c4d")
            w1c4ssb = cload(w1c4s_d, [4, L * P], dt.float32, "w1c4s")
            b2bcsb = cload(b2bc_d, [P, L * P], dt.float32, "b2bc")
            nw1hsb = cload(nw1h_d, [P, L * P], dt.bfloat16, "nw1h")
            nw1asb = cload(nw1a_d, [P, L * P], dt.bfloat16, "nw1a")
            nw2sb = cload(nw2_d, [P, L * P], dt.bfloat16, "nw2")
            nb1csb = cload(nb1c_d, [P, L], dt.float32, "nb1c")
            nb2bcsb = cload(nb2bc_d, [P, L * P], dt.float32, "nb2bc")
            lngbcsb = cload(lngbc_d, [P, L * P], dt.float32, "lngbc")
            lnbbcsb = cload(lnbbc_d, [P, L * P], dt.float32, "lnbbc")

            def emit_silu(out_ap, in_ap, scratch_pool, tag, bias=0.0):
                if not SILU_DECOMPOSED:
                    nc.scalar.activation(out_ap, in_ap, AF.Silu, bias=bias)
                else:
                    sg = scratch_pool.tile(
                        [P, in_ap.shape[-1] if in_ap.ndim == 2 else P],
                        dt.float32, tag=tag)
                    sga = sg[:in_ap.shape[0], :in_ap.shape[-1]]
                    nc.scalar.activation(sga, in_ap, AF.Sigmoid, bias=bias)
                    nc.vector.tensor_tensor(out=out_ap, in0=in_ap, in1=sga,
                                            op=ALU.mult)

            def edge_chunk(l, g0, g1, h1d_dram, h1s_dram, agg_hbm):
                lsl = slice(l * P, (l + 1) * P)
                if True:
                    nrows = (g1 - g0) * P
                    # dst node rows for these groups (contiguous, node-major)
                    nodes = gpool.tile([P, GCH * P], dt.bfloat16, tag="nodes")
                    nc.sync.dma_start(
                        out=nodes[:, :nrows].rearrange("p (g h) -> p g h", h=P),
                        in_=h1d_dram[g0 * P:g1 * P, :].rearrange(
                            "(g p) h -> p g h", p=P))
                    # per-edge slot ids along the free axis (for S_T build)
                    relf = gpool.tile([1, GCH * PPG * P], dt.bfloat16, tag="relf")
                    nc.sync.dma_start(
                        out=relf[:, :(g1 - g0) * PPG * P],
                        in_=dstrelF_d[:, g0 * PPG * P:g1 * PPG * P])
                    for g in range(g0, g1):
                        aggp = pagg.tile([P, P], dt.float32, tag="agg")
                        nodes_g = nodes[:, (g - g0) * P:(g - g0 + 1) * P]
                        for js in range(0, PPG, 4):
                            je = min(js + 4, PPG)
                            nsb = je - js
                            # src rows: one indirect gather per 128-edge panel
                            gs = gspool.tile([P, 4 * P], dt.bfloat16, tag="gs")
                            for k in range(nsb):
                                col = g * PPG + js + k
                                nc.gpsimd.indirect_dma_start(
                                    out=gs[:, k * P:(k + 1) * P],
                                    out_offset=None,
                                    in_=h1s_dram[:, :],
                                    in_offset=IOA(ap=srcidxsb[:, col:col + 1],
                                                  axis=0))
                            # S_T[slot, e] via ones-matmul broadcast + is_equal
                            rsl = slice(((g - g0) * PPG + js) * P,
                                        ((g - g0) * PPG + je) * P)
                            bcp = pmB.tile([P, 4 * P], dt.float32, tag="B")
                            nc.tensor.matmul(bcp[:, :nsb * P], lhsT=onesb[:],
                                             rhs=relf[:, rsl],
                                             start=True, stop=True)
                            ST = wpool.tile([P, 4 * P], dt.bfloat16, tag="ST")
                            nc.vector.tensor_scalar(
                                ST[:, :nsb * P], bcp[:, :nsb * P],
                                iotacol[:], None, op0=ALU.is_equal)
                            # m1t (feature-major, PSUM): gathered src rows are
                            # transposed in via matmul-with-identity, the dst
                            # expansion streams S_T against the node tile
                            m1tp = pmB.tile([P, 4 * P], dt.float32, tag="B")
                            for k in range(nsb):
                                ksl = slice(k * P, (k + 1) * P)
                                nc.tensor.matmul(
                                    m1tp[:, ksl], lhsT=gs[:, ksl],
                                    rhs=identsb[:], start=True, stop=False,
                                    skip_group_check=True)
                                nc.tensor.matmul(
                                    m1tp[:, ksl], lhsT=nodes_g,
                                    rhs=ST[:, ksl], start=False, stop=True,
                                    skip_group_check=True)
                            m1t = wpool.tile([P, 4 * P], dt.bfloat16, tag="m1tsb")
                            emit_silu(m1t[:, :nsb * P], m1tp[:, :nsb * P],
                                      wpool, "sg1")
                            m2p = pmB.tile([P, 4 * P], dt.float32, tag="B")
                            for k in range(nsb):
                                nc.tensor.matmul(
                                    m2p[:, k * P:(k + 1) * P],
                                    lhsT=m1t[:, k * P:(k + 1) * P],
                                    rhs=w2sb[:, lsl], start=True, stop=True)
                            nc.vector.tensor_tensor(
                                out=m2p[:, :nsb * P].rearrange(
                                    "p (j h) -> p j h", h=P),
                                in0=m2p[:, :nsb * P].rearrange(
                                    "p (j h) -> p j h", h=P),
                                in1=b2bcsb[:, lsl].unsqueeze(1)
                                    .to_broadcast([P, nsb, P]),
                                op=ALU.add)
                            m2s = wpool.tile([P, 4 * P], dt.bfloat16, tag="m2s")
                            emit_silu(m2s[:, :nsb * P], m2p[:, :nsb * P],
                                      wpool, "sg2")
                            S = wpool.tile([P, 4 * P], dt.bfloat16, tag="S")
                            csl = slice(g * PPG + js, g * PPG + je)
                            nc.vector.tensor_tensor(
                                out=S[:, :nsb * P].rearrange(
                                    "p (j s) -> p j s", s=P),
                                in0=dstrelsb[:, csl].unsqueeze(2)
                                    .to_broadcast([P, nsb, P]),
                                in1=iotasb[:].unsqueeze(1)
                                    .to_broadcast([P, nsb, P]),
                                op=ALU.is_equal)
                            for k in range(nsb):
                                nc.tensor.matmul(
                                    aggp[:], lhsT=S[:, k * P:(k + 1) * P],
                                    rhs=m2s[:, k * P:(k + 1) * P],
                                    start=(js == 0 and k == 0),
                                    stop=(je == PPG and k == nsb - 1),
                                    skip_group_check=True)
                        aggsb = spool.tile([P, P], dt.bfloat16, tag="aggsb")
                        nc.scalar.activation(aggsb[:], aggp[:], AF.Copy)
                        nc.sync.dma_start(out=agg_hbm[g * P:(g + 1) * P, :],
                                          in_=aggsb[:])

            def node_chunk(l, w0, w1, hprev, hnext, h1d_next, h1s_next,
                           agg_hbm):
                lsl = slice(l * P, (l + 1) * P)
                last = l == L - 1
                if True:
                    nw = w1 - w0
                    cnt = nw * P
                    rows = slice(w0 * P, w0 * P + cnt)
                    hwt = spool.tile([P, CW * P], dt.bfloat16, tag="hwt")
                    nc.sync.dma_start_transpose(hwt[:, :cnt], hprev[rows, :])
                    awt = spool.tile([P, CW * P], dt.bfloat16, tag="awt")
                    nc.sync.dma_start_transpose(awt[:, :cnt], agg_hbm[rows, :])
                    hwin = spool.tile([P, CW * P], dt.bfloat16, tag="hwin")
                    nc.sync.dma_start(
                        out=hwin[:, :cnt].rearrange("p (w h) -> p w h", h=P),
                        in_=hprev[rows, :].rearrange("(w p) h -> p w h", p=P))

                    n1p = pmB.tile([P, CW * P], dt.float32, tag="B")
                    nc.tensor.matmul(n1p[:, :cnt], lhsT=nw1hsb[:, lsl],
                                     rhs=hwt[:, :cnt], start=True, stop=False)
                    nc.tensor.matmul(n1p[:, :cnt], lhsT=nw1asb[:, lsl],
                                     rhs=awt[:, :cnt], start=False, stop=True)
                    n1s = spool.tile([P, CW * P], dt.bfloat16, tag="n1s")
                    emit_silu(n1s[:, :cnt], n1p[:, :cnt], spool, "sgn",
                              bias=nb1csb[:, l:l + 1])

                    n2p = pmB.tile([P, CW * P], dt.float32, tag="B")
                    for w in range(nw):
                        nc.tensor.matmul(n2p[:, w * P:(w + 1) * P],
                                         lhsT=n1s[:, w * P:(w + 1) * P],
                                         rhs=nw2sb[:, lsl],
                                         start=True, stop=True)
                    nc.vector.tensor_tensor(
                        out=n2p[:, :cnt].rearrange("p (w h) -> p w h", h=P),
                        in0=n2p[:, :cnt].rearrange("p (w h) -> p w h", h=P),
                        in1=nb2bcsb[:, lsl].unsqueeze(1)
                            .to_broadcast([P, nw, P]),
                        op=ALU.add)
                    nc.vector.tensor_tensor(
                        out=n2p[:, :cnt].rearrange("p (w h) -> p w h", h=P),
                        in0=n2p[:, :cnt].rearrange("p (w h) -> p w h", h=P),
                        in1=hwin[:, :cnt].rearrange("p (w h) -> p w h", h=P),
                        op=ALU.add)

                    mu = spool.tile([P, CW], dt.float32, tag="mu")
                    var = spool.tile([P, CW], dt.float32, tag="var")
                    sq = spool.tile([P, P], dt.float32, tag="sq")
                    for w in range(nw):
                        wsl = slice(w * P, (w + 1) * P)
                        nc.vector.reduce_sum(mu[:, w:w + 1], n2p[:, wsl],
                                             axis=AX.X)
                    nc.vector.tensor_scalar_mul(mu[:, :nw], mu[:, :nw], 1.0 / P)
                    for w in range(nw):
                        wsl = slice(w * P, (w + 1) * P)
                        nc.vector.tensor_scalar_sub(n2p[:, wsl], n2p[:, wsl],
                                                    mu[:, w:w + 1])
                        nc.scalar.activation(sq[:], n2p[:, wsl], AF.Square,
                                             accum_out=var[:, w:w + 1])
                    nc.vector.tensor_scalar(var[:, :nw], var[:, :nw],
                                            1.0 / P, EPS,
                                            op0=ALU.mult, op1=ALU.add)
                    srt = spool.tile([P, CW], dt.float32, tag="srt")
                    nc.scalar.activation(srt[:, :nw], var[:, :nw], AF.Sqrt)
                    rstd = spool.tile([P, CW], dt.float32, tag="rstd")
                    nc.vector.reciprocal(rstd[:, :nw], srt[:, :nw])

                    xn = spool.tile([P, CW * P], dt.float32, tag="xn")
                    for w in range(nw):
                        wsl = slice(w * P, (w + 1) * P)
                        nc.vector.tensor_scalar_mul(xn[:, wsl], n2p[:, wsl],
                                                    rstd[:, w:w + 1])
                    nc.vector.tensor_tensor(
                        out=xn[:, :cnt].rearrange("p (w h) -> p w h", h=P),
                        in0=xn[:, :cnt].rearrange("p (w h) -> p w h", h=P),
                        in1=lngbcsb[:, lsl].unsqueeze(1)
                            .to_broadcast([P, nw, P]),
                        op=ALU.mult)
                    nc.vector.tensor_tensor(
                        out=xn[:, :cnt].rearrange("p (w h) -> p w h", h=P),
                        in0=xn[:, :cnt].rearrange("p (w h) -> p w h", h=P),
                        in1=lnbbcsb[:, lsl].unsqueeze(1)
                            .to_broadcast([P, nw, P]),
                        op=ALU.add)

                    if last:
                        nc.sync.dma_start(
                            out=hout_d[rows, :].rearrange(
                                "(w p) h -> p w h", p=P),
                            in_=xn[:, :cnt].rearrange("p (w h) -> p w h", h=P))
                        return

                    hnb = spool.tile([P, CW * P], dt.bfloat16, tag="hnb")
                    nc.vector.tensor_copy(hnb[:, :cnt], xn[:, :cnt])
                    nc.sync.dma_start(
                        out=hnext[rows, :].rearrange("(w p) h -> p w h", p=P),
                        in_=hnb[:, :cnt].rearrange("p (w h) -> p w h", h=P))
                    hnt = spool.tile([P, CW * P], dt.bfloat16, tag="hnt")
                    nc.sync.dma_start_transpose(hnt[:, :cnt], hnext[rows, :])

                    nsl = slice((l + 1) * P, (l + 2) * P)
                    for (tbl, wab, w1c4b, tag) in (
                        (h1d_next, w1asb, w1c4dsb, "pd"),
                        (h1s_next, w1bsb, w1c4ssb, "ps"),
                    ):
                        pp = pmB.tile([P, CW * P], dt.float32, tag="B")
                        for w in range(nw):
                            wsl = slice(w * P, (w + 1) * P)
                            nc.tensor.matmul(pp[:, wsl],
                                             lhsT=hnt[:, wsl],
                                             rhs=wab[:, nsl],
                                             start=True, stop=False)
                            nc.tensor.matmul(
                                pp[:, wsl],
                                lhsT=x4sb[:, w0 * P + w * P:
                                          w0 * P + (w + 1) * P],
                                rhs=w1c4b[:, nsl],
                                start=False, stop=True)
                        pb = spool.tile([P, CW * P], dt.bfloat16, tag=tag)
                        nc.scalar.activation(pb[:, :cnt], pp[:, :cnt], AF.Copy)
                        nc.sync.dma_start(
                            out=tbl[rows, :].rearrange("(w p) h -> p w h", p=P),
                            in_=pb[:, :cnt].rearrange("p (w h) -> p w h", h=P))

            for l in range(L):
                h1d_dram = h1d0_d if l == 0 else h1down[l - 1]
                h1s_dram = h1sf0 if l == 0 else h1sfull[l - 1]
                hprev = h0own if l == 0 else hown[l - 1]
                agg = agg_hbm[l % 2]
                nxt = (hown[l], h1down[l], h1sown[l]) if l < L - 1 else \
                    (None, None, None)
                for g0 in range(0, ngrp, GCH):
                    g1 = min(g0 + GCH, ngrp)
                    edge_chunk(l, g0, g1, h1d_dram, h1s_dram, agg)
                    for w0 in range(g0, g1, CW):
                        w1 = min(w0 + CW, g1)
                        node_chunk(l, w0, w1, hprev, *nxt, agg)
                if l < L - 1:
                    nc.gpsimd.collective_compute(
                        "AllGather", mybir.AluOpType.bypass,
                        replica_groups=groups,
                        ins=[h1sown[l][:, :]], outs=[h1sfull[l][:, :]])

    nc.finalize()
    return nc


# ----------------------------------------------------------------------------
# Entry point
# ----------------------------------------------------------------------------

def _make_in_maps(geom, per_core, weights, host):
    in_maps = []
    for c in range(NCORES):
        pc = per_core[c]
        m = {
            "h1s_full0": host["h1s_full0"],
            "h1d_own0": pc["h1d_own0"],
            "h0own": pc["h0own"],
            "srcidx": pc["srcidx"], "dstrel": pc["dstrel"],
            "dstrelF": pc["dstrelF"], "x4own": pc["x4own"],
        }
        m.update(weights)
        in_maps.append(m)
    return in_maps


def _postprocess(geom, host, houts):
    nv = geom["nv"]
    h = np.concatenate([ho[:nv] for ho in houts], axis=0).astype(np.float32)
    v_pred = h @ host["op_w"] + host["op_b"]
    diff = v_pred - host["target"]
    return np.float32(np.mean(diff.astype(np.float64) ** 2))


def kernel(**inputs):
    from concourse.bass_utils import run_bass_kernel_spmd

    geom, per_core, weights, host = _preprocess(inputs)
    nc = _build_program(geom)
    in_maps = _make_in_maps(geom, per_core, weights, host)
    res = run_bass_kernel_spmd(nc, in_maps, list(range(NCORES)))
    houts = [res.results[c]["hout"] for c in range(NCORES)]
    return _postprocess(geom, host, houts)
